# revision 2
# baseline (speedup 1.0000x reference)
"""Trainium2 Bass kernel v2: GAT message passing (2 edge sets) + GRUCell + LayerNorm.

Key changes vs v1:
- Host pre-combines edge weight+mask: wc = w if adj else -1 (halves score DMA,
  removes all gpsimd adds, removes the fp16 saturation hack).
- Attention bias B'[j,(h,i)] = a_cur[i,h]+ba[h]+a_nb[j,h] built per j-tile by ONE
  k=5 PE matmul (ones/anbT rows x B_row/headmask), consumed directly from PSUM.
- ONE fused DVE score op per (set, j-tile) covering all 4 heads:
    u = select(wc < 0, -60000, leaky_relu(B' * wc, 0.2))
  (wc broadcast across heads via a stride-0 page dim). 32 calls instead of 128.
- exp batched per 2 j-tiles on ACT; msg PSUM->SBUF copies moved to gpsimd;
  softmax-normalize multiply moved to gpsimd; GRU bias row folded on host;
  rsqrt via ACT table switch instead of 5 Newton iterations on DVE.
- U matmuls emitted one 2-tile group late so PE never waits on ACT/DVE.
"""

import numpy as np

import concourse.bass as bass
import concourse.mybir as mybir
from concourse import bacc
import concourse.tile as tile
from concourse.bass_utils import run_bass_kernel_spmd

N, D, DH, H = 2048, 256, 256, 4
DHEAD = DH // H
NCORES = 8
S = N // NCORES          # 256 targets per core
JT = N // 128            # 16 j-tiles
KT = D // 128            # 2 k-tiles over d
F16 = mybir.dt.float16
F32 = mybir.dt.float32
AF = mybir.ActivationFunctionType
ALU = mybir.AluOpType

LAST_EXEC_NS = None

# ---------------------------------------------------------------- custom DVE op
_GATB_OP = None


def _register_gatb():
    """u = select(wc < 0, C0, leaky_relu(B' * wc, C1)).
    in0 = wc [P, 4(bcast), 256] fp16, in1 = B' [P, 4, 256] f32 (PSUM),
    s0 = mask value (-60000), s1 = leaky slope (0.2)."""
    global _GATB_OP
    if _GATB_OP is not None:
        return _GATB_OP
    import concourse.dve_ops as dve_ops
    from concourse.dve_spec import (
        C0, C1, Spec, Src0, Src1, Zero, _has_src1, lower as spec_lower,
        maxx, select,
    )
    from concourse.dve_uop import DveOpSpec

    name = "GATB_SCORE_ANT"
    for op in dve_ops.OPS:
        if op.name == name:
            _GATB_OP = op
            return op

    _q = Src0 * Src1
    body = select(Src0 < Zero, C0, maxx(_q, _q * C1))

    def _ref(in0, in1, s0, s1, imm2=None):
        q = in0.astype(np.float32) * in1.astype(np.float32)
        lr = np.maximum(q, q * np.float32(s1))
        return np.where(in0.astype(np.float32) < 0.0, np.float32(s0), lr).astype(
            np.float32
        )

    spec = Spec(body=body, reference=_ref)
    row = dve_ops._CUSTOM_DVE_ROW_BASE + len(dve_ops.OPS)
    shas = {}
    for ver in ("v3", "v4"):
        try:
            uops = spec_lower(spec, ver=ver)
            shas[ver] = DveOpSpec(
                name=name, opcode=row, uops=uops, rd1_en=_has_src1(spec)
            ).sha(ver)
        except Exception:
            pass
    op = dve_ops.DveOp(name, spec, subdim=False, uops_sha=shas,
                       perf_en={"v3": True, "v4": True})
    dve_ops.OPS.append(op)
    dve_ops.CUSTOM_DVE_SPECS[name] = spec
    dve_ops._SUB_OPCODE_FOR_NAME[name] = row
    _GATB_OP = op
    return op


# ---------------------------------------------------------------- bass program
_NC_CACHE = None


def _build_nc(dbg=False):
    global _NC_CACHE
    if _NC_CACHE is not None:
        return _NC_CACHE
    gatb = _register_gatb()

    nc = bacc.Bacc("TRN2", target_bir_lowering=False, debug=False,
                   enable_asserts=False)

    def din(nm, shape, dt):
        return nc.dram_tensor(nm, list(shape), dt, kind="ExternalInput").ap()

    wp = [din(f"wp{e}", (128, JT * S), F16) for e in range(2)]
    xT_d = din("xT", (D, N), F16)
    xisl_d = din("xisl", (D, S), F16)
    wiT_d = din("wiT", (8 * 65, 3 * D), F16)   # [520, 768] (zero row per piece)
    whT_d = din("whT", (D, 3 * D), F16)        # [256, 768]
    WmT_d = din("WmT", (D, 2 * DH), F16)       # [256, 512] (set0|set1 cols)
    WaTnb_d = din("WaTnb", (D, 8), F16)
    WaTcur_d = din("WaTcur", (D, 8), F16)
    ba_col_d = din("ba_col", (8, 1), F32)
    biasr_d = din("biasr", (1, 3 * D), F16)    # bih + bm_cat @ wih^T (host)
    bhhr_d = din("bhhr", (1, 3 * D), F16)
    hmask_d = din("hmask", (4, 4 * S), F16)    # hmask[h', h*S+i] = (h'==h)
    lnG_d = din("lnG", (128, D), F32)
    lnB_d = din("lnB", (128, D), F32)
    ones_d = din("ones", (1, 128), F16)
    ones2k_d = din("ones2k", (1, N), F16)
    onesf_d = din("onesf", (1, 64), F32)
    ident_d = din("ident", (128, 128), F16)
    onecol_d = din("onecol", (128, 128), F16)
    zcol_d = din("zcol", (1, 65), F16)

    out_d = nc.dram_tensor("out", [S, D], F32, kind="ExternalOutput").ap()
    dbg_d = {}
    if dbg:
        for nm, shape in [("d_Bp0", (128, 1024)), ("d_u0", (128, 1024)),
                          ("d_et0", (128, 1024)), ("d_anbT", (8, 2048)),
                          ("d_R0", (5, 1024)), ("d_aTcB", (8, 256)),
                          ("d_U00", (65, 256)), ("d_piece0", (64, 256)),
                          ("d_gh0", (128, 768)), ("d_gi0", (128, 768)),
                          ("d_hh0", (128, 256))]:
            dbg_d[nm] = nc.dram_tensor(nm, list(shape), F32,
                                       kind="ExternalOutput").ap()

    with tile.TileContext(nc) as tc:
        with (
            tc.tile_pool(name="const", bufs=1) as cp,
            tc.tile_pool(name="stream", bufs=1) as sp,
            tc.tile_pool(name="work", bufs=3) as wkp,
            tc.tile_pool(name="msg", bufs=1) as mp,
        ):
            def ddump(nm, ap):
                if not dbg or nm not in dbg_d:
                    return
                t = cp.tile(list(dbg_d[nm].shape), F32, tag=nm, name=nm)
                nc.vector.tensor_copy(t[:], ap)
                nc.sync.dma_start(out=dbg_d[nm][:, :], in_=t[:])

            def load(pool, nm, src, shape, dt, tag=None):
                t = pool.tile(shape, dt, tag=tag or nm, name=tag or nm)
                nc.sync.dma_start(out=t[:], in_=src)
                return t

            # ---------------- constants into SBUF
            wsb = [load(sp, f"wsb{e}", wp[e][:, :], [128, JT * S], F16)
                   for e in range(2)]
            xT = [load(cp, f"xT{k}", xT_d[128 * k:128 * (k + 1), :],
                       [128, N], F16) for k in range(KT)]
            xisl = [load(cp, f"xisl{k}", xisl_d[128 * k:128 * (k + 1), :],
                        [128, S], F16) for k in range(KT)]
            wiT = [load(cp, f"wiT{p}", wiT_d[65 * p:65 * (p + 1), :],
                        [65, 3 * D], F16) for p in range(8)]
            whT = [load(cp, f"whT{k}", whT_d[128 * k:128 * (k + 1), :],
                        [128, 3 * D], F16) for k in range(KT)]
            WmT = [load(cp, f"WmT{k}", WmT_d[128 * k:128 * (k + 1), :],
                        [128, 2 * DH], F16) for k in range(KT)]
            WaTnb = [load(cp, f"WaTnb{k}", WaTnb_d[128 * k:128 * (k + 1), :],
                          [128, 8], F16) for k in range(KT)]
            WaTcur = [load(cp, f"WaTcur{k}", WaTcur_d[128 * k:128 * (k + 1), :],
                           [128, 8], F16) for k in range(KT)]
            ba_col = load(cp, "ba_col", ba_col_d[:, :], [8, 1], F32)
            biasr = load(cp, "biasr", biasr_d[:, :], [1, 3 * D], F16)
            bhhr = load(cp, "bhhr", bhhr_d[:, :], [1, 3 * D], F16)
            lnG = load(cp, "lnG", lnG_d[:, :], [128, D], F32)
            lnB = load(cp, "lnB", lnB_d[:, :], [128, D], F32)
            ones = load(cp, "ones", ones_d[:, :], [1, 128], F16)
            zcol = load(cp, "zcol", zcol_d[:, :], [1, 65], F16)
            ones2k = load(cp, "ones2k", ones2k_d[:, :], [1, N], F16)
            onesf = load(cp, "onesf", onesf_d[:, :], [1, 64], F32)
            ident = load(cp, "ident", ident_d[:, :], [128, 128], F16)

            # R_e = [B_row_e(filled later); headmask]  [5, 4S] fp16
            R = []
            for e in range(2):
                r = cp.tile([5, 4 * S], F16, tag=f"R{e}", name=f"R{e}")
                nc.sync.dma_start(out=r[1:5, :], in_=hmask_d[:, :])
                R.append(r)
            # L_e = [ones_row; anbT_e(filled later)]  [5, N] fp16
            L = []
            for e in range(2):
                l_ = cp.tile([5, N], F16, tag=f"L{e}", name=f"L{e}")
                nc.sync.dma_start(out=l_[0:1, :], in_=ones2k_d[:, :])
                L.append(l_)

            # me tiles, ones in column 64
            me_all = cp.tile([128, JT, 2, 4, 66], F16, tag="me", name="me")
            nc.sync.dma_start(
                out=me_all[:, :, :, :, 0:1].rearrange("p a b c d -> p (a b c d)"),
                in_=onecol_d[:, :])

            # ---------------- prep: anbT, aTcB, xn
            psP_cm = tc.tile_pool(name="psP", bufs=2, space="PSUM")
            psP = psP_cm.__enter__()

            anbT8 = cp.tile([8, N], F16, tag="anbT8", name="anbT8")
            for c in range(4):
                csl = slice(512 * c, 512 * (c + 1))
                ps = psP.tile([8, 512], F32, tag="ps_p", name="ps_anbT")
                for k in range(KT):
                    nc.tensor.matmul(ps[:], WaTnb[k][:], xT[k][:, csl],
                                     start=(k == 0), stop=(k == KT - 1))
                if c % 2 == 0:
                    nc.vector.tensor_copy(anbT8[:, csl], ps[:])
                else:
                    nc.scalar.copy(anbT8[:, csl], ps[:])
            ddump("d_anbT", anbT8[:, :])
            for e in range(2):
                nc.sync.dma_start(out=L[e][1:5, :], in_=anbT8[4 * e:4 * e + 4, :])

            # a_cur^T + ba -> aTcB [8, S]; rows (4e+h) -> R_e row 0
            aTcB = cp.tile([8, S], F16, tag="aTcB", name="aTcB")
            for ih in range(2):
                ps = psP.tile([128, 8], F32, tag="ps_p", name="ps_ac")
                for k in range(KT):
                    nc.tensor.matmul(ps[:], xisl[k][:, 128 * ih:128 * (ih + 1)],
                                     WaTcur[k][:], start=(k == 0),
                                     stop=(k == KT - 1))
                ac = wkp.tile([128, 8], F16, tag="acur", name="acur")
                nc.vector.tensor_copy(ac[:], ps[:])
                pst = psP.tile([8, 128], F16, tag="ps_p2", name="ps_at")
                nc.tensor.transpose(pst[:], ac[:], ident[:])
                nc.vector.tensor_scalar_add(aTcB[:, 128 * ih:128 * (ih + 1)],
                                            pst[:], ba_col[:])
            ddump("d_aTcB", aTcB[:])
            for e in range(2):
                for h in range(4):
                    nc.sync.dma_start(out=R[e][0:1, S * h:S * (h + 1)],
                                      in_=aTcB[4 * e + h:4 * e + h + 1, :])
            ddump("d_R0", R[0][:, :])

            # x islice natural layout [i, d] fp16 (for the GRU h-path)
            xn = []
            for ih in range(2):
                t = cp.tile([128, D], F16, tag=f"xn{ih}", name=f"xn{ih}")
                for k in range(KT):
                    pst = psP.tile([128, 128], F16, tag="ps_p2", name="ps_xt")
                    nc.tensor.transpose(
                        pst[:], xisl[k][:, 128 * ih:128 * (ih + 1)], ident[:])
                    nc.scalar.copy(t[:, 128 * k:128 * (k + 1)], pst[:])
                xn.append(t)
            # msg matmuls for ALL tiles, both sets (merged moving side)
            for t in range(JT):
                tsl = slice(128 * t, 128 * (t + 1))
                psm = psP.tile([128, 2, 4, DHEAD], F32, tag="ps_m",
                               name="ps_m", bufs=4)
                for k in range(KT):
                    nc.tensor.matmul(psm[:], xT[k][:, tsl], WmT[k][:],
                                     start=(k == 0), stop=(k == KT - 1))
                if t % 2 == 0:
                    nc.vector.tensor_copy(me_all[:, t, :, :, 1:65], psm[:])
                else:
                    nc.scalar.copy(me_all[:, t, :, :, 1:65], psm[:])
            psP_cm.__exit__(None, None, None)

            # ---------------- main: scores, exp, msg, aggregation
            psB_cm = tc.tile_pool(name="psB", bufs=3, space="PSUM")
            psU_cm = tc.tile_pool(name="psU", bufs=1, space="PSUM")
            psB = psB_cm.__enter__(); psU = psU_cm.__enter__()

            msgT = []
            for e in range(2):
                U = psU.tile([65, 4, S], F32, tag="ps_U", name="ps_U")
                for zb in range(2):
                    nc.tensor.matmul(
                        U[:, 2 * zb:2 * zb + 2, :].rearrange("p a b -> p (a b)"),
                        zcol[:], ones2k[0:1, 0:2 * S], start=True, stop=False,
                        skip_group_check=True)
                pend = []   # U-matmul groups not yet emitted
                for t in range(JT):
                    sl = slice(S * t, S * (t + 1))
                    tsl = slice(128 * t, 128 * (t + 1))
                    # B'[j,(h,i)] via one k=5 matmul
                    psb = psB.tile([128, 4, S], F32, tag="ps_B", name="ps_B")
                    for bh in range(2):
                        nc.tensor.matmul(
                            psb[:, 2 * bh:2 * bh + 2, :].rearrange(
                                "p a b -> p (a b)"),
                            L[e][:, tsl], R[e][:, 2 * S * bh:2 * S * (bh + 1)],
                            start=True, stop=True)
                    if e == 0 and t == 0:
                        ddump("d_Bp0", psb[:].rearrange("p a b -> p (a b)"))
                    # fused masked leaky score, all 4 heads in one call
                    if t % 2 == 0:
                        u2 = wkp.tile([128, 2, 4, S], F16, tag="u", name="u",
                                      bufs=2)
                    wc3 = (wsb[e][:, sl]
                           .rearrange("p (o n) -> p o n", o=1)
                           .broadcast_to([128, 4, S]))
                    nc.vector._custom_dve(
                        gatb,
                        out=u2[:, t % 2],
                        in0=wc3,
                        in1=psb[:],
                        s0=-60000.0,
                        s1=0.2,
                    )
                    if e == 0 and t == 0:
                        ddump("d_u0", u2[:, 0].rearrange("p a b -> p (a b)"))
                    if t % 2 == 1:
                        et2 = wkp.tile([128, 2, 4, S], F16, tag="et", name="et",
                                       bufs=2)
                        nc.scalar.activation(et2[:], u2[:], AF.Exp)
                        if e == 0 and t == 1:
                            ddump("d_et0", et2[:, 0].rearrange("p a b -> p (a b)"))
                        pend.append((t - 1, t, et2))
                        # emit the PREVIOUS group's U matmuls (keeps PE ahead)
                        if len(pend) == 2:
                            ta, tb, pet = pend.pop(0)
                            for tt in (ta, tb):
                                for h in range(4):
                                    nc.tensor.matmul(
                                        U[:, h, :], me_all[:, tt, e, h, 0:65],
                                        pet[:, tt % 2, h],
                                        start=False,
                                        stop=(tt == JT - 1 and h % 2 == 1),
                                        skip_group_check=True)
                for ta, tb, pet in pend:
                    for tt in (ta, tb):
                        for h in range(4):
                            nc.tensor.matmul(U[:, h, :], me_all[:, tt, e, h, 0:65],
                                             pet[:, tt % 2, h],
                                             start=False,
                                             stop=(tt == JT - 1 and h % 2 == 1),
                                             skip_group_check=True)

                if e == 0:
                    ddump("d_U00", U[:, 0, :])
                # normalize: piece = U[0:64] / U[64]
                for h in range(4):
                    rd = wkp.tile([1, S], F32, tag="rd", name="rd")
                    nc.vector.reciprocal_approx_fast(rd[0:1, :], U[0:1, h, :])
                    rb = wkp.tile([65, S], F32, tag="rb", name="rb", bufs=2)
                    nc.gpsimd.partition_broadcast(rb[:], rd[0:1, :])
                    piece = mp.tile([65, S], F16, tag=f"msgT{4 * e + h}",
                                    name=f"msgT{4 * e + h}")
                    nc.vector.tensor_tensor(piece[:], U[:, h, :], rb[:],
                                            ALU.mult)
                    if e == 0 and h == 0:
                        ddump("d_piece0", piece[1:65, :])
                    msgT.append(piece)

            # ---------------- GRU per i-half (psums borrow the psB buffers)
            hhs = []
            for ih in range(2):
                ihs = slice(128 * ih, 128 * (ih + 1))
                # gh = x @ whh^T + bhh
                psgh = psB.tile([128, 4, S], F32, tag="ps_B",
                                name="ps_gh")[:, :, :].rearrange(
                                    "p a b -> p (a b)")[:, 0:3 * D]
                for lo, hi in ((0, 512), (512, 768)):
                    for k in range(KT):
                        nc.tensor.matmul(psgh[:, lo:hi], xisl[k][:, ihs],
                                         whT[k][:, lo:hi], start=(k == 0),
                                         stop=False)
                    nc.tensor.matmul(psgh[:, lo:hi], ones[:], bhhr[:, lo:hi],
                                     start=False, stop=True)
                gh = wkp.tile([128, 3 * D], F32, tag="gh", name="gh", bufs=2)
                nc.scalar.copy(gh[:], psgh[:])
                if ih == 0:
                    ddump("d_gh0", gh[:])

                # gi = msgcat @ wih^T + (bih + bm@wihT)
                psgi = psB.tile([128, 4, S], F32, tag="ps_B",
                                name="ps_gi")[:, :, :].rearrange(
                                    "p a b -> p (a b)")[:, 0:3 * D]
                for lo, hi in ((0, 512), (512, 768)):
                    for p in range(8):
                        nc.tensor.matmul(psgi[:, lo:hi], msgT[p][:, ihs],
                                         wiT[p][:, lo:hi], start=(p == 0),
                                         stop=False)
                    nc.tensor.matmul(psgi[:, lo:hi], ones[:], biasr[:, lo:hi],
                                     start=False, stop=True)
                if ih == 0:
                    ddump("d_gi0", psgi[:, :])

                # r/z = sigmoid(gi+gh) = 0.5*tanh(0.5*(gi+gh)) + 0.5 ; n = tanh
                trz = wkp.tile([128, 2 * D], F32, tag="trz", name="trz", bufs=2)
                nc.vector.tensor_tensor(trz[:], psgi[:, 0:2 * D], gh[:, 0:2 * D],
                                        ALU.add)
                th = wkp.tile([128, 2 * D], F32, tag="th", name="th", bufs=2)
                nc.scalar.activation(th[:], trz[:], AF.Tanh, scale=0.5)
                rz = wkp.tile([128, 2 * D], F32, tag="rz", name="rz", bufs=2)
                nc.vector.tensor_scalar(rz[:], th[:], 0.5, 0.5, ALU.mult,
                                        ALU.add)
                t1 = wkp.tile([128, D], F32, tag="t1", name="t1", bufs=2)
                nc.gpsimd.tensor_mul(t1[:], rz[:, 0:D], gh[:, 2 * D:3 * D])
                t2 = wkp.tile([128, D], F32, tag="t2", name="t2", bufs=2)
                nc.vector.tensor_tensor(t2[:], t1[:], psgi[:, 2 * D:3 * D],
                                        ALU.add)
                nn_ = wkp.tile([128, D], F32, tag="nn", name="nn", bufs=2)
                nc.scalar.activation(nn_[:], t2[:], AF.Tanh)
                # h = n + z*(x - n)
                t3 = wkp.tile([128, D], F32, tag="t3", name="t3", bufs=2)
                nc.gpsimd.tensor_sub(t3[:], xn[ih][:], nn_[:])
                t4 = wkp.tile([128, D], F32, tag="t4", name="t4", bufs=2)
                nc.gpsimd.tensor_mul(t4[:], t3[:], rz[:, D:2 * D])
                hh = wkp.tile([128, D], F32, tag="hh", name="hh", bufs=2)
                nc.vector.tensor_tensor(hh[:], nn_[:], t4[:], ALU.add)
                if ih == 0:
                    ddump("d_hh0", hh[:])
                hhs.append(hh)

            # ---------------- LayerNorm per i-half (single sqrt table load)
            for ih in range(2):
                ihs = slice(128 * ih, 128 * (ih + 1))
                hh = hhs[ih]
                st = wkp.tile([128, 6], F32, tag="st", name="st", bufs=2)
                nc.vector.bn_stats(out=st[:], in_=hh[:])
                mv = wkp.tile([128, 2], F32, tag="mv", name="mv", bufs=2)
                nc.vector.bn_aggr(out=mv[:], in_=st[:])
                veps = wkp.tile([128, 1], F32, tag="veps", name="veps", bufs=2)
                nc.vector.tensor_scalar_add(veps[:], mv[:, 1:2], 1e-5)
                rcp = wkp.tile([128, 1], F32, tag="rcp", name="rcp", bufs=2)
                nc.vector.reciprocal(rcp[:], veps[:])
                rv = wkp.tile([128, 1], F32, tag="rv", name="rv", bufs=2)
                nc.scalar.activation(rv[:], rcp[:], AF.Sqrt)
                hn = wkp.tile([128, D], F32, tag="hn", name="hn", bufs=2)
                nc.vector.tensor_scalar(hn[:], hh[:], mv[:, 0:1], rv[:],
                                        ALU.subtract, ALU.mult)
                og = wkp.tile([128, D], F32, tag="og", name="og", bufs=2)
                nc.gpsimd.tensor_mul(og[:], hn[:], lnG[:])
                ob = wkp.tile([128, D], F32, tag="ob", name="ob", bufs=2)
                nc.vector.tensor_tensor(ob[:], og[:], lnB[:], ALU.add)
                nc.sync.dma_start(out=out_d[ihs, :], in_=ob[:])
            psU_cm.__exit__(None, None, None)
            psB_cm.__exit__(None, None, None)

    nc.compile()
    _NC_CACHE = nc
    return nc


# ---------------------------------------------------------------- host wrapper
def _pack_tiles(a):
    """[N, S] -> [128, JT*S]: row-tile t, partition p holds a[t*128+p, :] at
    cols [t*S:(t+1)*S]."""
    n, s = a.shape
    t = n // 128
    return np.ascontiguousarray(
        a.reshape(t, 128, s).transpose(1, 0, 2).reshape(128, t * s))


def kernel(_dbg=False, **inputs):
    global LAST_EXEC_NS
    f16 = np.float16
    x = np.asarray(inputs["axiom_states"], np.float32)
    adj = [np.asarray(inputs["adj0"], np.float32),
           np.asarray(inputs["adj1"], np.float32)]
    w = [np.asarray(inputs["w0"], np.float32),
         np.asarray(inputs["w1"], np.float32)]
    Wm = [np.asarray(inputs["Wm0"], np.float32),
          np.asarray(inputs["Wm1"], np.float32)]
    bm = [np.asarray(inputs["bm0"], np.float32),
          np.asarray(inputs["bm1"], np.float32)]
    Wa = [np.asarray(inputs["Wa0"], np.float32),
          np.asarray(inputs["Wa1"], np.float32)]
    ba = [np.asarray(inputs["ba0"], np.float32),
          np.asarray(inputs["ba1"], np.float32)]
    wih = np.asarray(inputs["gru_wih"], np.float32)
    whh = np.asarray(inputs["gru_whh"], np.float32)
    bih = np.asarray(inputs["gru_bih"], np.float32)
    bhh = np.asarray(inputs["gru_bhh"], np.float32)
    ln_g = np.asarray(inputs["ln_g"], np.float32)
    ln_b = np.asarray(inputs["ln_b"], np.float32)

    xT = np.ascontiguousarray(x.T).astype(f16)                     # [256, 2048]
    wiTraw = np.ascontiguousarray(wih.T).astype(np.float32)        # [512, 768]
    wiT = np.zeros((8 * 65, 768), np.float32)
    for p in range(8):
        wiT[65 * p + 1:65 * (p + 1)] = wiTraw[64 * p:64 * (p + 1)]
    wiT = wiT.astype(f16)
    whT = np.ascontiguousarray(whh.T).astype(f16)                  # [256, 768]
    WmT = np.concatenate([Wm[0].T, Wm[1].T], 1).astype(f16)        # [256, 512]
    WaTnb = np.concatenate([Wa[0][:, D:].T, Wa[1][:, D:].T], 1).astype(f16)
    WaTcur = np.concatenate([Wa[0][:, :D].T, Wa[1][:, :D].T], 1).astype(f16)
    ba_col = np.concatenate([ba[0], ba[1]]).reshape(8, 1).astype(np.float32)
    bm_cat = np.concatenate([bm[0], bm[1]])                        # [512]
    biasr = (bih + bm_cat @ wih.T).reshape(1, -1).astype(f16)      # [1, 768]
    bhhr = bhh.reshape(1, -1).astype(f16)
    hmask = np.zeros((4, 4 * S), np.float32)
    for h in range(4):
        hmask[h, S * h:S * (h + 1)] = 1.0
    hmask = hmask.astype(f16)
    lnG = np.broadcast_to(ln_g, (128, D)).astype(np.float32).copy()
    lnB = np.broadcast_to(ln_b, (128, D)).astype(np.float32).copy()

    wc = [np.where(adj[e] != 0.0, w[e], -1.0).astype(f16) for e in range(2)]

    nc = _build_nc(dbg=_dbg)

    in_maps = []
    for c in range(NCORES):
        isl = slice(c * S, (c + 1) * S)
        m = {
            "wp0": _pack_tiles(wc[0][:, isl]),
            "wp1": _pack_tiles(wc[1][:, isl]),
            "xT": xT,
            "xisl": np.ascontiguousarray(xT[:, isl]),
            "wiT": wiT, "whT": whT, "WmT": WmT,
            "WaTnb": WaTnb, "WaTcur": WaTcur, "ba_col": ba_col,
            "biasr": biasr, "bhhr": bhhr, "hmask": hmask,
            "lnG": lnG, "lnB": lnB,
            "ones": np.ones((1, 128), f16),
            "ones2k": np.ones((1, N), f16),
            "onesf": np.ones((1, 64), np.float32),
            "ident": np.eye(128, dtype=f16),
            "onecol": np.ones((128, 128), f16),
            "zcol": np.zeros((1, 65), f16),
        }
        in_maps.append(m)

    import os
    trace = bool(int(os.environ.get("KERNEL_TRACE", "0")))
    if trace:
        try:
            import axon_ntff_shim  # noqa: F401  (registers the NTFF hook)
        except ImportError:
            trace = False
    res = run_bass_kernel_spmd(nc, in_maps, core_ids=list(range(NCORES)),
                               trace=trace)
    LAST_EXEC_NS = res.exec_time_ns
    out = np.concatenate([r["out"] for r in res.results], axis=0)
    if _dbg:
        global LAST_DBG
        LAST_DBG = res.results
    return out


# revision 3
# speedup vs baseline: 1.0400x; 1.0400x over previous
"""Trainium2 Bass kernel v2: GAT message passing (2 edge sets) + GRUCell + LayerNorm.

Key changes vs v1:
- Host pre-combines edge weight+mask: wc = w if adj else -1 (halves score DMA,
  removes all gpsimd adds, removes the fp16 saturation hack).
- Attention bias B'[j,(h,i)] = a_cur[i,h]+ba[h]+a_nb[j,h] built per j-tile by ONE
  k=5 PE matmul (ones/anbT rows x B_row/headmask), consumed directly from PSUM.
- ONE fused DVE score op per (set, j-tile) covering all 4 heads:
    u = select(wc < 0, -60000, leaky_relu(B' * wc, 0.2))
  (wc broadcast across heads via a stride-0 page dim). 32 calls instead of 128.
- exp batched per 2 j-tiles on ACT; msg PSUM->SBUF copies moved to gpsimd;
  softmax-normalize multiply moved to gpsimd; GRU bias row folded on host;
  rsqrt via ACT table switch instead of 5 Newton iterations on DVE.
- U matmuls emitted one 2-tile group late so PE never waits on ACT/DVE.
"""

import numpy as np

import concourse.bass as bass
import concourse.mybir as mybir
from concourse import bacc
import concourse.tile as tile
from concourse.bass_utils import run_bass_kernel_spmd

N, D, DH, H = 2048, 256, 256, 4
DHEAD = DH // H
NCORES = 8
S = N // NCORES          # 256 targets per core
JT = N // 128            # 16 j-tiles
KT = D // 128            # 2 k-tiles over d
F16 = mybir.dt.float16
F32 = mybir.dt.float32
AF = mybir.ActivationFunctionType
ALU = mybir.AluOpType

LAST_EXEC_NS = None

# ---------------------------------------------------------------- custom DVE op
_GATB_OP = None


def _register_gatb():
    """u = select(wc < 0, C0, leaky_relu(B' * wc, C1)).
    in0 = wc [P, 4(bcast), 256] fp16, in1 = B' [P, 4, 256] f32 (PSUM),
    s0 = mask value (-60000), s1 = leaky slope (0.2)."""
    global _GATB_OP
    if _GATB_OP is not None:
        return _GATB_OP
    import concourse.dve_ops as dve_ops
    from concourse.dve_spec import (
        C0, C1, Spec, Src0, Src1, Zero, _has_src1, lower as spec_lower,
        maxx, select,
    )
    from concourse.dve_uop import DveOpSpec

    name = "GATB_SCORE_ANT"
    for op in dve_ops.OPS:
        if op.name == name:
            _GATB_OP = op
            return op

    _q = Src0 * Src1
    body = select(Src0 < Zero, C0, maxx(_q, _q * C1))

    def _ref(in0, in1, s0, s1, imm2=None):
        q = in0.astype(np.float32) * in1.astype(np.float32)
        lr = np.maximum(q, q * np.float32(s1))
        return np.where(in0.astype(np.float32) < 0.0, np.float32(s0), lr).astype(
            np.float32
        )

    spec = Spec(body=body, reference=_ref)
    row = dve_ops._CUSTOM_DVE_ROW_BASE + len(dve_ops.OPS)
    shas = {}
    for ver in ("v3", "v4"):
        try:
            uops = spec_lower(spec, ver=ver)
            shas[ver] = DveOpSpec(
                name=name, opcode=row, uops=uops, rd1_en=_has_src1(spec)
            ).sha(ver)
        except Exception:
            pass
    op = dve_ops.DveOp(name, spec, subdim=False, uops_sha=shas,
                       perf_en={"v3": True, "v4": True})
    dve_ops.OPS.append(op)
    dve_ops.CUSTOM_DVE_SPECS[name] = spec
    dve_ops._SUB_OPCODE_FOR_NAME[name] = row
    _GATB_OP = op
    return op


# ---------------------------------------------------------------- bass program
_NC_CACHE = None


def _build_nc(dbg=False):
    global _NC_CACHE
    if _NC_CACHE is not None:
        return _NC_CACHE
    gatb = _register_gatb()

    nc = bacc.Bacc("TRN2", target_bir_lowering=False, debug=False,
                   enable_asserts=False)

    def din(nm, shape, dt):
        return nc.dram_tensor(nm, list(shape), dt, kind="ExternalInput").ap()

    wp = [din(f"wp{e}", (128, JT * S), F16) for e in range(2)]
    xT_d = din("xT", (D, N), F16)
    xisl_d = din("xisl", (D, S), F16)
    wiT_d = din("wiT", (8 * 65, 3 * D), F16)   # [520, 768] (zero row per piece)
    whT_d = din("whT", (D, 3 * D), F16)        # [256, 768]
    WmT_d = din("WmT", (D, 2 * DH), F16)       # [256, 512] (set0|set1 cols)
    WaTnb_d = din("WaTnb", (D, 8), F16)
    WaTcur_d = din("WaTcur", (D, 8), F16)
    ba_col_d = din("ba_col", (8, 1), F32)
    biasr_d = din("biasr", (1, 3 * D), F16)    # bih + bm_cat @ wih^T (host)
    bhhr_d = din("bhhr", (1, 3 * D), F16)
    hmask_d = din("hmask", (4, 4 * S), F16)    # hmask[h', h*S+i] = (h'==h)
    lnG_d = din("lnG", (128, D), F32)
    lnB_d = din("lnB", (128, D), F32)
    ones_d = din("ones", (1, 128), F16)
    ones2k_d = din("ones2k", (1, N), F16)
    onesf_d = din("onesf", (1, 64), F32)
    ident_d = din("ident", (128, 128), F16)
    onecol_d = din("onecol", (128, 128), F16)
    zcol_d = din("zcol", (1, 65), F16)

    out_d = nc.dram_tensor("out", [S, D], F32, kind="ExternalOutput").ap()
    dbg_d = {}
    if dbg:
        for nm, shape in [("d_Bp0", (128, 1024)), ("d_u0", (128, 1024)),
                          ("d_et0", (128, 1024)), ("d_anbT", (8, 2048)),
                          ("d_R0", (5, 1024)), ("d_aTcB", (8, 256)),
                          ("d_U00", (65, 256)), ("d_piece0", (64, 256)),
                          ("d_gh0", (128, 768)), ("d_gi0", (128, 768)),
                          ("d_hh0", (128, 256))]:
            dbg_d[nm] = nc.dram_tensor(nm, list(shape), F32,
                                       kind="ExternalOutput").ap()

    with tile.TileContext(nc) as tc:
        with (
            tc.tile_pool(name="const", bufs=1) as cp,
            tc.tile_pool(name="stream", bufs=1) as sp,
            tc.tile_pool(name="work", bufs=3) as wkp,
            tc.tile_pool(name="msg", bufs=1) as mp,
        ):
            def ddump(nm, ap):
                if not dbg or nm not in dbg_d:
                    return
                t = cp.tile(list(dbg_d[nm].shape), F32, tag=nm, name=nm)
                nc.vector.tensor_copy(t[:], ap)
                nc.sync.dma_start(out=dbg_d[nm][:, :], in_=t[:])

            def load(pool, nm, src, shape, dt, tag=None):
                t = pool.tile(shape, dt, tag=tag or nm, name=tag or nm)
                nc.sync.dma_start(out=t[:], in_=src)
                return t

            # ---------------- constants into SBUF (emission order = DMA order)
            xT = [load(cp, f"xT{k}", xT_d[128 * k:128 * (k + 1), :],
                       [128, N], F16) for k in range(KT)]
            xisl = [load(cp, f"xisl{k}", xisl_d[128 * k:128 * (k + 1), :],
                        [128, S], F16) for k in range(KT)]
            WaTnb = [load(cp, f"WaTnb{k}", WaTnb_d[128 * k:128 * (k + 1), :],
                          [128, 8], F16) for k in range(KT)]
            WaTcur = [load(cp, f"WaTcur{k}", WaTcur_d[128 * k:128 * (k + 1), :],
                           [128, 8], F16) for k in range(KT)]
            wsb = [sp.tile([128, JT * S], F16, tag=f"wsb{e}", name=f"wsb{e}")
                   for e in range(2)]
            nc.sync.dma_start(out=wsb[0][:], in_=wp[0][:, :])
            WmT = [load(cp, f"WmT{k}", WmT_d[128 * k:128 * (k + 1), :],
                        [128, 2 * DH], F16) for k in range(KT)]
            wiT = [load(cp, f"wiT{p}", wiT_d[65 * p:65 * (p + 1), :],
                        [65, 3 * D], F16) for p in range(8)]
            whT = [load(cp, f"whT{k}", whT_d[128 * k:128 * (k + 1), :],
                        [128, 3 * D], F16) for k in range(KT)]
            ba_col = load(cp, "ba_col", ba_col_d[:, :], [8, 1], F32)
            biasr = load(cp, "biasr", biasr_d[:, :], [1, 3 * D], F16)
            bhhr = load(cp, "bhhr", bhhr_d[:, :], [1, 3 * D], F16)
            lnG = load(cp, "lnG", lnG_d[:, :], [128, D], F32)
            lnB = load(cp, "lnB", lnB_d[:, :], [128, D], F32)
            ones = load(cp, "ones", ones_d[:, :], [1, 128], F16)
            zcol = load(cp, "zcol", zcol_d[:, :], [1, 65], F16)
            ones2k = load(cp, "ones2k", ones2k_d[:, :], [1, N], F16)
            onesf = load(cp, "onesf", onesf_d[:, :], [1, 64], F32)
            ident = load(cp, "ident", ident_d[:, :], [128, 128], F16)
            nc.sync.dma_start(out=wsb[1][:], in_=wp[1][:, :])

            # R_e = [B_row_e(filled later); headmask]  [5, 4S] fp16
            R = []
            for e in range(2):
                r = cp.tile([5, 4 * S], F16, tag=f"R{e}", name=f"R{e}")
                nc.gpsimd.dma_start(out=r[1:5, :], in_=hmask_d[:, :])
                R.append(r)
            # L_e = [ones_row; anbT_e(filled later)]  [5, N] fp16
            L = []
            for e in range(2):
                l_ = cp.tile([5, N], F16, tag=f"L{e}", name=f"L{e}")
                nc.gpsimd.dma_start(out=l_[0:1, :], in_=ones2k_d[:, :])
                L.append(l_)

            # me tiles, ones in column 64
            me_all = cp.tile([128, JT, 2, 4, 66], F16, tag="me", name="me")
            nc.sync.dma_start(
                out=me_all[:, :, :, :, 0:1].rearrange("p a b c d -> p (a b c d)"),
                in_=onecol_d[:, :])

            # ---------------- prep: anbT, aTcB, xn
            psP_cm = tc.tile_pool(name="psP", bufs=2, space="PSUM")
            psP = psP_cm.__enter__()

            anbT8 = cp.tile([8, N], F16, tag="anbT8", name="anbT8")
            for c in range(4):
                csl = slice(512 * c, 512 * (c + 1))
                ps = psP.tile([8, 512], F32, tag="ps_p", name="ps_anbT")
                for k in range(KT):
                    nc.tensor.matmul(ps[:], WaTnb[k][:], xT[k][:, csl],
                                     start=(k == 0), stop=(k == KT - 1))
                if c % 2 == 0:
                    nc.vector.tensor_copy(anbT8[:, csl], ps[:])
                else:
                    nc.scalar.copy(anbT8[:, csl], ps[:])
            ddump("d_anbT", anbT8[:, :])
            for e in range(2):
                nc.gpsimd.dma_start(out=L[e][1:5, :], in_=anbT8[4 * e:4 * e + 4, :])

            # a_cur^T + ba -> aTcB [8, S]; rows (4e+h) -> R_e row 0
            aTcB = cp.tile([8, S], F16, tag="aTcB", name="aTcB")
            for ih in range(2):
                ps = psP.tile([128, 8], F32, tag="ps_p", name="ps_ac")
                for k in range(KT):
                    nc.tensor.matmul(ps[:], xisl[k][:, 128 * ih:128 * (ih + 1)],
                                     WaTcur[k][:], start=(k == 0),
                                     stop=(k == KT - 1))
                ac = wkp.tile([128, 8], F16, tag="acur", name="acur")
                nc.vector.tensor_copy(ac[:], ps[:])
                pst = psP.tile([8, 128], F16, tag="ps_p2", name="ps_at")
                nc.tensor.transpose(pst[:], ac[:], ident[:])
                nc.vector.tensor_scalar_add(aTcB[:, 128 * ih:128 * (ih + 1)],
                                            pst[:], ba_col[:])
            ddump("d_aTcB", aTcB[:])
            for e in range(2):
                for h in range(4):
                    nc.gpsimd.dma_start(out=R[e][0:1, S * h:S * (h + 1)],
                                        in_=aTcB[4 * e + h:4 * e + h + 1, :])
            ddump("d_R0", R[0][:, :])

            # x islice natural layout [i, d] fp16 (for the GRU h-path)
            xn = []
            for ih in range(2):
                t = cp.tile([128, D], F16, tag=f"xn{ih}", name=f"xn{ih}")
                for k in range(KT):
                    pst = psP.tile([128, 128], F16, tag="ps_p2", name="ps_xt")
                    nc.tensor.transpose(
                        pst[:], xisl[k][:, 128 * ih:128 * (ih + 1)], ident[:])
                    nc.scalar.copy(t[:, 128 * k:128 * (k + 1)], pst[:])
                xn.append(t)
            # msg matmuls for ALL tiles, both sets (merged moving side)
            for t in range(JT):
                tsl = slice(128 * t, 128 * (t + 1))
                psm = psP.tile([128, 2, 4, DHEAD], F32, tag="ps_m",
                               name="ps_m", bufs=4)
                for k in range(KT):
                    nc.tensor.matmul(psm[:], xT[k][:, tsl], WmT[k][:],
                                     start=(k == 0), stop=(k == KT - 1))
                if t % 2 == 0:
                    nc.vector.tensor_copy(me_all[:, t, :, :, 1:65], psm[:])
                else:
                    nc.scalar.copy(me_all[:, t, :, :, 1:65], psm[:])
            psP_cm.__exit__(None, None, None)

            # ---------------- main: scores, exp, msg, aggregation
            psB_cm = tc.tile_pool(name="psB", bufs=3, space="PSUM")
            psU_cm = tc.tile_pool(name="psU", bufs=1, space="PSUM")
            psB = psB_cm.__enter__(); psU = psU_cm.__enter__()

            msgT = []
            for e in range(2):
                U = psU.tile([65, 4, S], F32, tag="ps_U", name="ps_U")
                for zb in range(2):
                    nc.tensor.matmul(
                        U[:, 2 * zb:2 * zb + 2, :].rearrange("p a b -> p (a b)"),
                        zcol[:], ones2k[0:1, 0:2 * S], start=True, stop=False,
                        skip_group_check=True)
                pend = []   # U-matmul groups not yet emitted
                for t in range(JT):
                    sl = slice(S * t, S * (t + 1))
                    tsl = slice(128 * t, 128 * (t + 1))
                    # B'[j,(h,i)] via one k=5 matmul
                    psb = psB.tile([128, 4, S], F32, tag="ps_B", name="ps_B")
                    for bh in range(2):
                        nc.tensor.matmul(
                            psb[:, 2 * bh:2 * bh + 2, :].rearrange(
                                "p a b -> p (a b)"),
                            L[e][:, tsl], R[e][:, 2 * S * bh:2 * S * (bh + 1)],
                            start=True, stop=True)
                    if e == 0 and t == 0:
                        ddump("d_Bp0", psb[:].rearrange("p a b -> p (a b)"))
                    # fused masked leaky score, all 4 heads in one call
                    if t % 2 == 0:
                        u2 = wkp.tile([128, 2, 4, S], F16, tag="u", name="u",
                                      bufs=2)
                    wc3 = (wsb[e][:, sl]
                           .rearrange("p (o n) -> p o n", o=1)
                           .broadcast_to([128, 4, S]))
                    nc.vector._custom_dve(
                        gatb,
                        out=u2[:, t % 2],
                        in0=wc3,
                        in1=psb[:],
                        s0=-60000.0,
                        s1=0.2,
                    )
                    if e == 0 and t == 0:
                        ddump("d_u0", u2[:, 0].rearrange("p a b -> p (a b)"))
                    if t % 2 == 1:
                        et2 = wkp.tile([128, 2, 4, S], F16, tag="et", name="et",
                                       bufs=2)
                        nc.scalar.activation(et2[:], u2[:], AF.Exp)
                        if e == 0 and t == 1:
                            ddump("d_et0", et2[:, 0].rearrange("p a b -> p (a b)"))
                        pend.append((t - 1, t, et2))
                        # emit the PREVIOUS group's U matmuls (keeps PE ahead)
                        if len(pend) == 2:
                            ta, tb, pet = pend.pop(0)
                            for tt in (ta, tb):
                                for h in range(4):
                                    nc.tensor.matmul(
                                        U[:, h, :], me_all[:, tt, e, h, 0:65],
                                        pet[:, tt % 2, h],
                                        start=False,
                                        stop=(tt == JT - 1 and h % 2 == 1),
                                        skip_group_check=True)
                for ta, tb, pet in pend:
                    for tt in (ta, tb):
                        for h in range(4):
                            nc.tensor.matmul(U[:, h, :], me_all[:, tt, e, h, 0:65],
                                             pet[:, tt % 2, h],
                                             start=False,
                                             stop=(tt == JT - 1 and h % 2 == 1),
                                             skip_group_check=True)

                if e == 0:
                    ddump("d_U00", U[:, 0, :])
                # normalize: piece = U[0:64] / U[64]
                for h in range(4):
                    rd = wkp.tile([1, S], F32, tag="rd", name="rd")
                    nc.vector.reciprocal_approx_fast(rd[0:1, :], U[0:1, h, :])
                    rb = wkp.tile([65, S], F32, tag="rb", name="rb", bufs=2)
                    nc.gpsimd.partition_broadcast(rb[:], rd[0:1, :])
                    piece = mp.tile([65, S], F16, tag=f"msgT{4 * e + h}",
                                    name=f"msgT{4 * e + h}")
                    nc.vector.tensor_tensor(piece[:], U[:, h, :], rb[:],
                                            ALU.mult)
                    if e == 0 and h == 0:
                        ddump("d_piece0", piece[1:65, :])
                    msgT.append(piece)

            # ---------------- GRU per i-half (psums borrow the psB buffers)
            hhs = []
            for ih in range(2):
                ihs = slice(128 * ih, 128 * (ih + 1))
                # gh = x @ whh^T + bhh
                psgh = psB.tile([128, 4, S], F32, tag="ps_B",
                                name="ps_gh")[:, :, :].rearrange(
                                    "p a b -> p (a b)")[:, 0:3 * D]
                for lo, hi in ((0, 512), (512, 768)):
                    for k in range(KT):
                        nc.tensor.matmul(psgh[:, lo:hi], xisl[k][:, ihs],
                                         whT[k][:, lo:hi], start=(k == 0),
                                         stop=False)
                    nc.tensor.matmul(psgh[:, lo:hi], ones[:], bhhr[:, lo:hi],
                                     start=False, stop=True)
                gh = wkp.tile([128, 3 * D], F32, tag="gh", name="gh", bufs=2)
                nc.scalar.copy(gh[:], psgh[:])
                if ih == 0:
                    ddump("d_gh0", gh[:])

                # gi = msgcat @ wih^T + (bih + bm@wihT)
                psgi = psB.tile([128, 4, S], F32, tag="ps_B",
                                name="ps_gi")[:, :, :].rearrange(
                                    "p a b -> p (a b)")[:, 0:3 * D]
                for lo, hi in ((0, 512), (512, 768)):
                    for p in range(8):
                        nc.tensor.matmul(psgi[:, lo:hi], msgT[p][:, ihs],
                                         wiT[p][:, lo:hi], start=(p == 0),
                                         stop=False)
                    nc.tensor.matmul(psgi[:, lo:hi], ones[:], biasr[:, lo:hi],
                                     start=False, stop=True)
                if ih == 0:
                    ddump("d_gi0", psgi[:, :])

                # r/z = sigmoid(gi+gh) = 0.5*tanh(0.5*(gi+gh)) + 0.5 ; n = tanh
                trz = wkp.tile([128, 2 * D], F32, tag="trz", name="trz", bufs=2)
                nc.vector.tensor_tensor(trz[:], psgi[:, 0:2 * D], gh[:, 0:2 * D],
                                        ALU.add)
                th = wkp.tile([128, 2 * D], F32, tag="th", name="th", bufs=2)
                nc.scalar.activation(th[:], trz[:], AF.Tanh, scale=0.5)
                rz = wkp.tile([128, 2 * D], F32, tag="rz", name="rz", bufs=2)
                nc.vector.tensor_scalar(rz[:], th[:], 0.5, 0.5, ALU.mult,
                                        ALU.add)
                t1 = wkp.tile([128, D], F32, tag="t1", name="t1", bufs=2)
                nc.gpsimd.tensor_mul(t1[:], rz[:, 0:D], gh[:, 2 * D:3 * D])
                t2 = wkp.tile([128, D], F32, tag="t2", name="t2", bufs=2)
                nc.vector.tensor_tensor(t2[:], t1[:], psgi[:, 2 * D:3 * D],
                                        ALU.add)
                nn_ = wkp.tile([128, D], F32, tag="nn", name="nn", bufs=2)
                nc.scalar.activation(nn_[:], t2[:], AF.Tanh)
                # h = n + z*(x - n)
                t3 = wkp.tile([128, D], F32, tag="t3", name="t3", bufs=2)
                nc.gpsimd.tensor_sub(t3[:], xn[ih][:], nn_[:])
                t4 = wkp.tile([128, D], F32, tag="t4", name="t4", bufs=2)
                nc.gpsimd.tensor_mul(t4[:], t3[:], rz[:, D:2 * D])
                hh = wkp.tile([128, D], F32, tag="hh", name="hh", bufs=2)
                nc.vector.tensor_tensor(hh[:], nn_[:], t4[:], ALU.add)
                if ih == 0:
                    ddump("d_hh0", hh[:])
                hhs.append(hh)

            # ---------------- LayerNorm per i-half (single sqrt table load)
            for ih in range(2):
                ihs = slice(128 * ih, 128 * (ih + 1))
                hh = hhs[ih]
                st = wkp.tile([128, 6], F32, tag="st", name="st", bufs=2)
                nc.vector.bn_stats(out=st[:], in_=hh[:])
                mv = wkp.tile([128, 2], F32, tag="mv", name="mv", bufs=2)
                nc.vector.bn_aggr(out=mv[:], in_=st[:])
                veps = wkp.tile([128, 1], F32, tag="veps", name="veps", bufs=2)
                nc.vector.tensor_scalar_add(veps[:], mv[:, 1:2], 1e-5)
                rcp = wkp.tile([128, 1], F32, tag="rcp", name="rcp", bufs=2)
                nc.vector.reciprocal(rcp[:], veps[:])
                rv = wkp.tile([128, 1], F32, tag="rv", name="rv", bufs=2)
                nc.scalar.activation(rv[:], rcp[:], AF.Sqrt)
                hn = wkp.tile([128, D], F32, tag="hn", name="hn", bufs=2)
                nc.vector.tensor_scalar(hn[:], hh[:], mv[:, 0:1], rv[:],
                                        ALU.subtract, ALU.mult)
                og = wkp.tile([128, D], F32, tag="og", name="og", bufs=2)
                nc.gpsimd.tensor_mul(og[:], hn[:], lnG[:])
                ob = wkp.tile([128, D], F32, tag="ob", name="ob", bufs=2)
                nc.vector.tensor_tensor(ob[:], og[:], lnB[:], ALU.add)
                nc.sync.dma_start(out=out_d[ihs, :], in_=ob[:])
            psU_cm.__exit__(None, None, None)
            psB_cm.__exit__(None, None, None)

    nc.compile()
    _NC_CACHE = nc
    return nc


# ---------------------------------------------------------------- host wrapper
def _pack_tiles(a):
    """[N, S] -> [128, JT*S]: row-tile t, partition p holds a[t*128+p, :] at
    cols [t*S:(t+1)*S]."""
    n, s = a.shape
    t = n // 128
    return np.ascontiguousarray(
        a.reshape(t, 128, s).transpose(1, 0, 2).reshape(128, t * s))


def kernel(_dbg=False, **inputs):
    global LAST_EXEC_NS
    f16 = np.float16
    x = np.asarray(inputs["axiom_states"], np.float32)
    adj = [np.asarray(inputs["adj0"], np.float32),
           np.asarray(inputs["adj1"], np.float32)]
    w = [np.asarray(inputs["w0"], np.float32),
         np.asarray(inputs["w1"], np.float32)]
    Wm = [np.asarray(inputs["Wm0"], np.float32),
          np.asarray(inputs["Wm1"], np.float32)]
    bm = [np.asarray(inputs["bm0"], np.float32),
          np.asarray(inputs["bm1"], np.float32)]
    Wa = [np.asarray(inputs["Wa0"], np.float32),
          np.asarray(inputs["Wa1"], np.float32)]
    ba = [np.asarray(inputs["ba0"], np.float32),
          np.asarray(inputs["ba1"], np.float32)]
    wih = np.asarray(inputs["gru_wih"], np.float32)
    whh = np.asarray(inputs["gru_whh"], np.float32)
    bih = np.asarray(inputs["gru_bih"], np.float32)
    bhh = np.asarray(inputs["gru_bhh"], np.float32)
    ln_g = np.asarray(inputs["ln_g"], np.float32)
    ln_b = np.asarray(inputs["ln_b"], np.float32)

    xT = np.ascontiguousarray(x.T).astype(f16)                     # [256, 2048]
    wiTraw = np.ascontiguousarray(wih.T).astype(np.float32)        # [512, 768]
    wiT = np.zeros((8 * 65, 768), np.float32)
    for p in range(8):
        wiT[65 * p + 1:65 * (p + 1)] = wiTraw[64 * p:64 * (p + 1)]
    wiT = wiT.astype(f16)
    whT = np.ascontiguousarray(whh.T).astype(f16)                  # [256, 768]
    WmT = np.concatenate([Wm[0].T, Wm[1].T], 1).astype(f16)        # [256, 512]
    WaTnb = np.concatenate([Wa[0][:, D:].T, Wa[1][:, D:].T], 1).astype(f16)
    WaTcur = np.concatenate([Wa[0][:, :D].T, Wa[1][:, :D].T], 1).astype(f16)
    ba_col = np.concatenate([ba[0], ba[1]]).reshape(8, 1).astype(np.float32)
    bm_cat = np.concatenate([bm[0], bm[1]])                        # [512]
    biasr = (bih + bm_cat @ wih.T).reshape(1, -1).astype(f16)      # [1, 768]
    bhhr = bhh.reshape(1, -1).astype(f16)
    hmask = np.zeros((4, 4 * S), np.float32)
    for h in range(4):
        hmask[h, S * h:S * (h + 1)] = 1.0
    hmask = hmask.astype(f16)
    lnG = np.broadcast_to(ln_g, (128, D)).astype(np.float32).copy()
    lnB = np.broadcast_to(ln_b, (128, D)).astype(np.float32).copy()

    wc = [np.where(adj[e] != 0.0, w[e], -1.0).astype(f16) for e in range(2)]

    nc = _build_nc(dbg=_dbg)

    in_maps = []
    for c in range(NCORES):
        isl = slice(c * S, (c + 1) * S)
        m = {
            "wp0": _pack_tiles(wc[0][:, isl]),
            "wp1": _pack_tiles(wc[1][:, isl]),
            "xT": xT,
            "xisl": np.ascontiguousarray(xT[:, isl]),
            "wiT": wiT, "whT": whT, "WmT": WmT,
            "WaTnb": WaTnb, "WaTcur": WaTcur, "ba_col": ba_col,
            "biasr": biasr, "bhhr": bhhr, "hmask": hmask,
            "lnG": lnG, "lnB": lnB,
            "ones": np.ones((1, 128), f16),
            "ones2k": np.ones((1, N), f16),
            "onesf": np.ones((1, 64), np.float32),
            "ident": np.eye(128, dtype=f16),
            "onecol": np.ones((128, 128), f16),
            "zcol": np.zeros((1, 65), f16),
        }
        in_maps.append(m)

    import os
    trace = bool(int(os.environ.get("KERNEL_TRACE", "0")))
    if trace:
        try:
            import axon_ntff_shim  # noqa: F401  (registers the NTFF hook)
        except ImportError:
            trace = False
    res = run_bass_kernel_spmd(nc, in_maps, core_ids=list(range(NCORES)),
                               trace=trace)
    LAST_EXEC_NS = res.exec_time_ns
    out = np.concatenate([r["out"] for r in res.results], axis=0)
    if _dbg:
        global LAST_DBG
        LAST_DBG = res.results
    return out


# revision 4
# speedup vs baseline: 1.0405x; 1.0005x over previous
"""Trainium2 Bass kernel v2: GAT message passing (2 edge sets) + GRUCell + LayerNorm.

Key changes vs v1:
- Host pre-combines edge weight+mask: wc = w if adj else -1 (halves score DMA,
  removes all gpsimd adds, removes the fp16 saturation hack).
- Attention bias B'[j,(h,i)] = a_cur[i,h]+ba[h]+a_nb[j,h] built per j-tile by ONE
  k=5 PE matmul (ones/anbT rows x B_row/headmask), consumed directly from PSUM.
- ONE fused DVE score op per (set, j-tile) covering all 4 heads:
    u = select(wc < 0, -60000, leaky_relu(B' * wc, 0.2))
  (wc broadcast across heads via a stride-0 page dim). 32 calls instead of 128.
- exp batched per 2 j-tiles on ACT; msg PSUM->SBUF copies moved to gpsimd;
  softmax-normalize multiply moved to gpsimd; GRU bias row folded on host;
  rsqrt via ACT table switch instead of 5 Newton iterations on DVE.
- U matmuls emitted one 2-tile group late so PE never waits on ACT/DVE.
"""

import numpy as np

import concourse.bass as bass
import concourse.mybir as mybir
from concourse import bacc
import concourse.tile as tile
from concourse.bass_utils import run_bass_kernel_spmd

N, D, DH, H = 2048, 256, 256, 4
DHEAD = DH // H
NCORES = 8
S = N // NCORES          # 256 targets per core
JT = N // 128            # 16 j-tiles
KT = D // 128            # 2 k-tiles over d
F16 = mybir.dt.float16
F32 = mybir.dt.float32
AF = mybir.ActivationFunctionType
ALU = mybir.AluOpType

LAST_EXEC_NS = None

# ---------------------------------------------------------------- custom DVE op
_GATB_OP = None


def _register_gatb():
    """u = select(wc < 0, C0, leaky_relu(B' * wc, C1)).
    in0 = wc [P, 4(bcast), 256] fp16, in1 = B' [P, 4, 256] f32 (PSUM),
    s0 = mask value (-60000), s1 = leaky slope (0.2)."""
    global _GATB_OP
    if _GATB_OP is not None:
        return _GATB_OP
    import concourse.dve_ops as dve_ops
    from concourse.dve_spec import (
        C0, C1, Spec, Src0, Src1, Zero, _has_src1, lower as spec_lower,
        maxx, select,
    )
    from concourse.dve_uop import DveOpSpec

    name = "GATB_SCORE_ANT"
    for op in dve_ops.OPS:
        if op.name == name:
            _GATB_OP = op
            return op

    _q = Src0 * Src1
    body = select(Src0 < Zero, C0, maxx(_q, _q * C1))

    def _ref(in0, in1, s0, s1, imm2=None):
        q = in0.astype(np.float32) * in1.astype(np.float32)
        lr = np.maximum(q, q * np.float32(s1))
        return np.where(in0.astype(np.float32) < 0.0, np.float32(s0), lr).astype(
            np.float32
        )

    spec = Spec(body=body, reference=_ref)
    row = dve_ops._CUSTOM_DVE_ROW_BASE + len(dve_ops.OPS)
    shas = {}
    for ver in ("v3", "v4"):
        try:
            uops = spec_lower(spec, ver=ver)
            shas[ver] = DveOpSpec(
                name=name, opcode=row, uops=uops, rd1_en=_has_src1(spec)
            ).sha(ver)
        except Exception:
            pass
    op = dve_ops.DveOp(name, spec, subdim=False, uops_sha=shas,
                       perf_en={"v3": True, "v4": True})
    dve_ops.OPS.append(op)
    dve_ops.CUSTOM_DVE_SPECS[name] = spec
    dve_ops._SUB_OPCODE_FOR_NAME[name] = row
    _GATB_OP = op
    return op


# ---------------------------------------------------------------- bass program
_NC_CACHE = None


def _build_nc(dbg=False):
    global _NC_CACHE
    if _NC_CACHE is not None:
        return _NC_CACHE
    gatb = _register_gatb()

    nc = bacc.Bacc("TRN2", target_bir_lowering=False, debug=False,
                   enable_asserts=False)

    def din(nm, shape, dt):
        return nc.dram_tensor(nm, list(shape), dt, kind="ExternalInput").ap()

    wp = [din(f"wp{e}", (128, JT * S), F16) for e in range(2)]
    xT_d = din("xT", (D, N), F16)
    xisl_d = din("xisl", (D, S), F16)
    wiT_d = din("wiT", (8 * 65, 3 * D), F16)   # [520, 768] (zero row per piece)
    whT_d = din("whT", (D, 3 * D), F16)        # [256, 768]
    WmT_d = din("WmT", (D, 2 * DH), F16)       # [256, 512] (set0|set1 cols)
    WaTnb_d = din("WaTnb", (D, 8), F16)
    WaTcur_d = din("WaTcur", (D, 8), F16)
    ba_col_d = din("ba_col", (8, 1), F32)
    biasr_d = din("biasr", (1, 3 * D), F16)    # bih + bm_cat @ wih^T (host)
    bhhr_d = din("bhhr", (1, 3 * D), F16)
    hmask_d = din("hmask", (4, 4 * S), F16)    # hmask[h', h*S+i] = (h'==h)
    lnG_d = din("lnG", (128, D), F32)
    lnB_d = din("lnB", (128, D), F32)
    ones_d = din("ones", (1, 128), F16)
    ones2k_d = din("ones2k", (1, N), F16)
    onesf_d = din("onesf", (1, 64), F32)
    ident_d = din("ident", (128, 128), F16)
    onecol_d = din("onecol", (128, 128), F16)
    zcol_d = din("zcol", (1, 65), F16)

    out_d = nc.dram_tensor("out", [S, D], F32, kind="ExternalOutput").ap()
    dbg_d = {}
    if dbg:
        for nm, shape in [("d_Bp0", (128, 1024)), ("d_u0", (128, 1024)),
                          ("d_et0", (128, 1024)), ("d_anbT", (8, 2048)),
                          ("d_R0", (5, 1024)), ("d_aTcB", (8, 256)),
                          ("d_U00", (65, 256)), ("d_piece0", (64, 256)),
                          ("d_gh0", (128, 768)), ("d_gi0", (128, 768)),
                          ("d_hh0", (128, 256))]:
            dbg_d[nm] = nc.dram_tensor(nm, list(shape), F32,
                                       kind="ExternalOutput").ap()

    with tile.TileContext(nc) as tc:
        with (
            tc.tile_pool(name="const", bufs=1) as cp,
            tc.tile_pool(name="stream", bufs=1) as sp,
            tc.tile_pool(name="work", bufs=3) as wkp,
            tc.tile_pool(name="msg", bufs=1) as mp,
        ):
            def ddump(nm, ap):
                if not dbg or nm not in dbg_d:
                    return
                t = cp.tile(list(dbg_d[nm].shape), F32, tag=nm, name=nm)
                nc.vector.tensor_copy(t[:], ap)
                nc.sync.dma_start(out=dbg_d[nm][:, :], in_=t[:])

            def load(pool, nm, src, shape, dt, tag=None):
                t = pool.tile(shape, dt, tag=tag or nm, name=tag or nm)
                nc.sync.dma_start(out=t[:], in_=src)
                return t

            # ---------------- constants into SBUF (emission order = DMA order)
            xT = [load(cp, f"xT{k}", xT_d[128 * k:128 * (k + 1), :],
                       [128, N], F16) for k in range(KT)]
            xisl = [load(cp, f"xisl{k}", xisl_d[128 * k:128 * (k + 1), :],
                        [128, S], F16) for k in range(KT)]
            WaTnb = [load(cp, f"WaTnb{k}", WaTnb_d[128 * k:128 * (k + 1), :],
                          [128, 8], F16) for k in range(KT)]
            WaTcur = [load(cp, f"WaTcur{k}", WaTcur_d[128 * k:128 * (k + 1), :],
                           [128, 8], F16) for k in range(KT)]
            wsb = [sp.tile([128, JT * S], F16, tag=f"wsb{e}", name=f"wsb{e}")
                   for e in range(2)]
            nc.sync.dma_start(out=wsb[0][:], in_=wp[0][:, :])
            WmT = [load(cp, f"WmT{k}", WmT_d[128 * k:128 * (k + 1), :],
                        [128, 2 * DH], F16) for k in range(KT)]
            wiT = [load(cp, f"wiT{p}", wiT_d[65 * p:65 * (p + 1), :],
                        [65, 3 * D], F16) for p in range(8)]
            whT = [load(cp, f"whT{k}", whT_d[128 * k:128 * (k + 1), :],
                        [128, 3 * D], F16) for k in range(KT)]
            ba_col = load(cp, "ba_col", ba_col_d[:, :], [8, 1], F32)
            biasr = load(cp, "biasr", biasr_d[:, :], [1, 3 * D], F16)
            bhhr = load(cp, "bhhr", bhhr_d[:, :], [1, 3 * D], F16)
            lnG = load(cp, "lnG", lnG_d[:, :], [128, D], F32)
            lnB = load(cp, "lnB", lnB_d[:, :], [128, D], F32)
            ones = load(cp, "ones", ones_d[:, :], [1, 128], F16)
            zcol = load(cp, "zcol", zcol_d[:, :], [1, 65], F16)
            ones2k = load(cp, "ones2k", ones2k_d[:, :], [1, N], F16)
            onesf = load(cp, "onesf", onesf_d[:, :], [1, 64], F32)
            ident = load(cp, "ident", ident_d[:, :], [128, 128], F16)
            nc.sync.dma_start(out=wsb[1][:], in_=wp[1][:, :])

            # R_e = [B_row_e(filled later); headmask]  [5, 4S] fp16
            R = []
            for e in range(2):
                r = cp.tile([5, 4 * S], F16, tag=f"R{e}", name=f"R{e}")
                nc.gpsimd.dma_start(out=r[1:5, :], in_=hmask_d[:, :])
                R.append(r)
            # L_e = [ones_row; anbT_e(filled later)]  [5, N] fp16
            L = []
            for e in range(2):
                l_ = cp.tile([5, N], F16, tag=f"L{e}", name=f"L{e}")
                nc.gpsimd.dma_start(out=l_[0:1, :], in_=ones2k_d[:, :])
                L.append(l_)

            # me tiles, ones in column 64
            me_all = cp.tile([128, JT, 2, 4, 66], F16, tag="me", name="me")
            nc.gpsimd.memset(
                me_all[:, :, :, :, 0:1].rearrange("p a b c d -> p (a b c d)"),
                1.0)

            # ---------------- prep: anbT, aTcB, xn
            psP_cm = tc.tile_pool(name="psP", bufs=2, space="PSUM")
            psP = psP_cm.__enter__()

            anbT8 = cp.tile([8, N], F16, tag="anbT8", name="anbT8")
            for c in range(4):
                csl = slice(512 * c, 512 * (c + 1))
                ps = psP.tile([8, 512], F32, tag="ps_p", name="ps_anbT")
                for k in range(KT):
                    nc.tensor.matmul(ps[:], WaTnb[k][:], xT[k][:, csl],
                                     start=(k == 0), stop=(k == KT - 1))
                if c % 2 == 0:
                    nc.vector.tensor_copy(anbT8[:, csl], ps[:])
                else:
                    nc.scalar.copy(anbT8[:, csl], ps[:])
            ddump("d_anbT", anbT8[:, :])
            for e in range(2):
                nc.gpsimd.dma_start(out=L[e][1:5, :], in_=anbT8[4 * e:4 * e + 4, :])

            # a_cur^T + ba -> aTcB [8, S]; rows (4e+h) -> R_e row 0
            aTcB = cp.tile([8, S], F16, tag="aTcB", name="aTcB")
            for ih in range(2):
                ps = psP.tile([128, 8], F32, tag="ps_p", name="ps_ac")
                for k in range(KT):
                    nc.tensor.matmul(ps[:], xisl[k][:, 128 * ih:128 * (ih + 1)],
                                     WaTcur[k][:], start=(k == 0),
                                     stop=(k == KT - 1))
                ac = wkp.tile([128, 8], F16, tag="acur", name="acur")
                nc.vector.tensor_copy(ac[:], ps[:])
                pst = psP.tile([8, 128], F16, tag="ps_p2", name="ps_at")
                nc.tensor.transpose(pst[:], ac[:], ident[:])
                nc.vector.tensor_scalar_add(aTcB[:, 128 * ih:128 * (ih + 1)],
                                            pst[:], ba_col[:])
            ddump("d_aTcB", aTcB[:])
            for e in range(2):
                for h in range(4):
                    nc.gpsimd.dma_start(out=R[e][0:1, S * h:S * (h + 1)],
                                        in_=aTcB[4 * e + h:4 * e + h + 1, :])
            ddump("d_R0", R[0][:, :])

            # x islice natural layout [i, d] fp16 (for the GRU h-path)
            xn = []
            for ih in range(2):
                t = cp.tile([128, D], F16, tag=f"xn{ih}", name=f"xn{ih}")
                for k in range(KT):
                    pst = psP.tile([128, 128], F16, tag="ps_p2", name="ps_xt")
                    nc.tensor.transpose(
                        pst[:], xisl[k][:, 128 * ih:128 * (ih + 1)], ident[:])
                    nc.scalar.copy(t[:, 128 * k:128 * (k + 1)], pst[:])
                xn.append(t)
            # msg matmuls for ALL tiles, both sets (merged moving side)
            for t in range(JT):
                tsl = slice(128 * t, 128 * (t + 1))
                psm = psP.tile([128, 2, 4, DHEAD], F32, tag="ps_m",
                               name="ps_m", bufs=4)
                for k in range(KT):
                    nc.tensor.matmul(psm[:], xT[k][:, tsl], WmT[k][:],
                                     start=(k == 0), stop=(k == KT - 1))
                if t % 2 == 0:
                    nc.vector.tensor_copy(me_all[:, t, :, :, 1:65], psm[:])
                else:
                    nc.scalar.copy(me_all[:, t, :, :, 1:65], psm[:])
            psP_cm.__exit__(None, None, None)

            # ---------------- main: scores, exp, msg, aggregation
            psB_cm = tc.tile_pool(name="psB", bufs=3, space="PSUM")
            psU_cm = tc.tile_pool(name="psU", bufs=1, space="PSUM")
            psB = psB_cm.__enter__(); psU = psU_cm.__enter__()

            msgT = []
            for e in range(2):
                U = psU.tile([65, 4, S], F32, tag="ps_U", name="ps_U")
                for zb in range(2):
                    nc.tensor.matmul(
                        U[:, 2 * zb:2 * zb + 2, :].rearrange("p a b -> p (a b)"),
                        zcol[:], ones2k[0:1, 0:2 * S], start=True, stop=False,
                        skip_group_check=True)
                pend = []   # U-matmul groups not yet emitted
                for t in range(JT):
                    sl = slice(S * t, S * (t + 1))
                    tsl = slice(128 * t, 128 * (t + 1))
                    # B'[j,(h,i)] via one k=5 matmul
                    psb = psB.tile([128, 4, S], F32, tag="ps_B", name="ps_B")
                    for bh in range(2):
                        nc.tensor.matmul(
                            psb[:, 2 * bh:2 * bh + 2, :].rearrange(
                                "p a b -> p (a b)"),
                            L[e][:, tsl], R[e][:, 2 * S * bh:2 * S * (bh + 1)],
                            start=True, stop=True)
                    if e == 0 and t == 0:
                        ddump("d_Bp0", psb[:].rearrange("p a b -> p (a b)"))
                    # fused masked leaky score, all 4 heads in one call
                    if t % 2 == 0:
                        u2 = wkp.tile([128, 2, 4, S], F16, tag="u", name="u",
                                      bufs=2)
                    wc3 = (wsb[e][:, sl]
                           .rearrange("p (o n) -> p o n", o=1)
                           .broadcast_to([128, 4, S]))
                    nc.vector._custom_dve(
                        gatb,
                        out=u2[:, t % 2],
                        in0=wc3,
                        in1=psb[:],
                        s0=-60000.0,
                        s1=0.2,
                    )
                    if e == 0 and t == 0:
                        ddump("d_u0", u2[:, 0].rearrange("p a b -> p (a b)"))
                    if t % 2 == 1:
                        et2 = wkp.tile([128, 2, 4, S], F16, tag="et", name="et",
                                       bufs=2)
                        nc.scalar.activation(et2[:], u2[:], AF.Exp)
                        if e == 0 and t == 1:
                            ddump("d_et0", et2[:, 0].rearrange("p a b -> p (a b)"))
                        pend.append((t - 1, t, et2))
                        # emit the PREVIOUS group's U matmuls (keeps PE ahead)
                        if len(pend) == 2:
                            ta, tb, pet = pend.pop(0)
                            for tt in (ta, tb):
                                for h in range(4):
                                    nc.tensor.matmul(
                                        U[:, h, :], me_all[:, tt, e, h, 0:65],
                                        pet[:, tt % 2, h],
                                        start=False,
                                        stop=(tt == JT - 1 and h % 2 == 1),
                                        skip_group_check=True)
                for ta, tb, pet in pend:
                    for tt in (ta, tb):
                        for h in range(4):
                            nc.tensor.matmul(U[:, h, :], me_all[:, tt, e, h, 0:65],
                                             pet[:, tt % 2, h],
                                             start=False,
                                             stop=(tt == JT - 1 and h % 2 == 1),
                                             skip_group_check=True)

                if e == 0:
                    ddump("d_U00", U[:, 0, :])
                # normalize: piece = U[0:64] / U[64]
                for h in range(4):
                    rd = wkp.tile([1, S], F32, tag="rd", name="rd")
                    nc.vector.reciprocal_approx_fast(rd[0:1, :], U[0:1, h, :])
                    rb = wkp.tile([65, S], F32, tag="rb", name="rb", bufs=2)
                    nc.gpsimd.partition_broadcast(rb[:], rd[0:1, :])
                    piece = mp.tile([65, S], F16, tag=f"msgT{4 * e + h}",
                                    name=f"msgT{4 * e + h}")
                    nc.vector.tensor_tensor(piece[:], U[:, h, :], rb[:],
                                            ALU.mult)
                    if e == 0 and h == 0:
                        ddump("d_piece0", piece[1:65, :])
                    msgT.append(piece)

            # ---------------- GRU per i-half (psums borrow the psB buffers)
            hhs = []
            for ih in range(2):
                ihs = slice(128 * ih, 128 * (ih + 1))
                # gh = x @ whh^T + bhh
                psgh = psB.tile([128, 4, S], F32, tag="ps_B",
                                name="ps_gh")[:, :, :].rearrange(
                                    "p a b -> p (a b)")[:, 0:3 * D]
                for lo, hi in ((0, 512), (512, 768)):
                    for k in range(KT):
                        nc.tensor.matmul(psgh[:, lo:hi], xisl[k][:, ihs],
                                         whT[k][:, lo:hi], start=(k == 0),
                                         stop=False)
                    nc.tensor.matmul(psgh[:, lo:hi], ones[:], bhhr[:, lo:hi],
                                     start=False, stop=True)
                gh = wkp.tile([128, 3 * D], F32, tag="gh", name="gh", bufs=2)
                nc.scalar.copy(gh[:], psgh[:])
                if ih == 0:
                    ddump("d_gh0", gh[:])

                # gi = msgcat @ wih^T + (bih + bm@wihT)
                psgi = psB.tile([128, 4, S], F32, tag="ps_B",
                                name="ps_gi")[:, :, :].rearrange(
                                    "p a b -> p (a b)")[:, 0:3 * D]
                for lo, hi in ((0, 512), (512, 768)):
                    for p in range(8):
                        nc.tensor.matmul(psgi[:, lo:hi], msgT[p][:, ihs],
                                         wiT[p][:, lo:hi], start=(p == 0),
                                         stop=False)
                    nc.tensor.matmul(psgi[:, lo:hi], ones[:], biasr[:, lo:hi],
                                     start=False, stop=True)
                if ih == 0:
                    ddump("d_gi0", psgi[:, :])

                # r/z = sigmoid(gi+gh) = 0.5*tanh(0.5*(gi+gh)) + 0.5 ; n = tanh
                trz = wkp.tile([128, 2 * D], F32, tag="trz", name="trz", bufs=2)
                nc.vector.tensor_tensor(trz[:], psgi[:, 0:2 * D], gh[:, 0:2 * D],
                                        ALU.add)
                th = wkp.tile([128, 2 * D], F32, tag="th", name="th", bufs=2)
                nc.scalar.activation(th[:], trz[:], AF.Tanh, scale=0.5)
                rz = wkp.tile([128, 2 * D], F32, tag="rz", name="rz", bufs=2)
                nc.vector.tensor_scalar(rz[:], th[:], 0.5, 0.5, ALU.mult,
                                        ALU.add)
                t1 = wkp.tile([128, D], F32, tag="t1", name="t1", bufs=2)
                nc.gpsimd.tensor_mul(t1[:], rz[:, 0:D], gh[:, 2 * D:3 * D])
                t2 = wkp.tile([128, D], F32, tag="t2", name="t2", bufs=2)
                nc.vector.tensor_tensor(t2[:], t1[:], psgi[:, 2 * D:3 * D],
                                        ALU.add)
                nn_ = wkp.tile([128, D], F32, tag="nn", name="nn", bufs=2)
                nc.scalar.activation(nn_[:], t2[:], AF.Tanh)
                # h = n + z*(x - n)
                t3 = wkp.tile([128, D], F32, tag="t3", name="t3", bufs=2)
                nc.gpsimd.tensor_sub(t3[:], xn[ih][:], nn_[:])
                t4 = wkp.tile([128, D], F32, tag="t4", name="t4", bufs=2)
                nc.gpsimd.tensor_mul(t4[:], t3[:], rz[:, D:2 * D])
                hh = wkp.tile([128, D], F32, tag="hh", name="hh", bufs=2)
                nc.vector.tensor_tensor(hh[:], nn_[:], t4[:], ALU.add)
                if ih == 0:
                    ddump("d_hh0", hh[:])
                hhs.append(hh)

            # ---------------- LayerNorm per i-half (single sqrt table load)
            for ih in range(2):
                ihs = slice(128 * ih, 128 * (ih + 1))
                hh = hhs[ih]
                st = wkp.tile([128, 6], F32, tag="st", name="st", bufs=2)
                nc.vector.bn_stats(out=st[:], in_=hh[:])
                mv = wkp.tile([128, 2], F32, tag="mv", name="mv", bufs=2)
                nc.vector.bn_aggr(out=mv[:], in_=st[:])
                veps = wkp.tile([128, 1], F32, tag="veps", name="veps", bufs=2)
                nc.vector.tensor_scalar_add(veps[:], mv[:, 1:2], 1e-5)
                rcp = wkp.tile([128, 1], F32, tag="rcp", name="rcp", bufs=2)
                nc.vector.reciprocal(rcp[:], veps[:])
                rv = wkp.tile([128, 1], F32, tag="rv", name="rv", bufs=2)
                nc.scalar.activation(rv[:], rcp[:], AF.Sqrt)
                hn = wkp.tile([128, D], F32, tag="hn", name="hn", bufs=2)
                nc.vector.tensor_scalar(hn[:], hh[:], mv[:, 0:1], rv[:],
                                        ALU.subtract, ALU.mult)
                og = wkp.tile([128, D], F32, tag="og", name="og", bufs=2)
                nc.gpsimd.tensor_mul(og[:], hn[:], lnG[:])
                ob = wkp.tile([128, D], F32, tag="ob", name="ob", bufs=2)
                nc.vector.tensor_tensor(ob[:], og[:], lnB[:], ALU.add)
                nc.sync.dma_start(out=out_d[ihs, :], in_=ob[:])
            psU_cm.__exit__(None, None, None)
            psB_cm.__exit__(None, None, None)

    nc.compile()
    _NC_CACHE = nc
    return nc


# ---------------------------------------------------------------- host wrapper
def _pack_tiles(a):
    """[N, S] -> [128, JT*S]: row-tile t, partition p holds a[t*128+p, :] at
    cols [t*S:(t+1)*S]."""
    n, s = a.shape
    t = n // 128
    return np.ascontiguousarray(
        a.reshape(t, 128, s).transpose(1, 0, 2).reshape(128, t * s))


def kernel(_dbg=False, **inputs):
    global LAST_EXEC_NS
    f16 = np.float16
    x = np.asarray(inputs["axiom_states"], np.float32)
    adj = [np.asarray(inputs["adj0"], np.float32),
           np.asarray(inputs["adj1"], np.float32)]
    w = [np.asarray(inputs["w0"], np.float32),
         np.asarray(inputs["w1"], np.float32)]
    Wm = [np.asarray(inputs["Wm0"], np.float32),
          np.asarray(inputs["Wm1"], np.float32)]
    bm = [np.asarray(inputs["bm0"], np.float32),
          np.asarray(inputs["bm1"], np.float32)]
    Wa = [np.asarray(inputs["Wa0"], np.float32),
          np.asarray(inputs["Wa1"], np.float32)]
    ba = [np.asarray(inputs["ba0"], np.float32),
          np.asarray(inputs["ba1"], np.float32)]
    wih = np.asarray(inputs["gru_wih"], np.float32)
    whh = np.asarray(inputs["gru_whh"], np.float32)
    bih = np.asarray(inputs["gru_bih"], np.float32)
    bhh = np.asarray(inputs["gru_bhh"], np.float32)
    ln_g = np.asarray(inputs["ln_g"], np.float32)
    ln_b = np.asarray(inputs["ln_b"], np.float32)

    xT = np.ascontiguousarray(x.T).astype(f16)                     # [256, 2048]
    wiTraw = np.ascontiguousarray(wih.T).astype(np.float32)        # [512, 768]
    wiT = np.zeros((8 * 65, 768), np.float32)
    for p in range(8):
        wiT[65 * p + 1:65 * (p + 1)] = wiTraw[64 * p:64 * (p + 1)]
    wiT = wiT.astype(f16)
    whT = np.ascontiguousarray(whh.T).astype(f16)                  # [256, 768]
    WmT = np.concatenate([Wm[0].T, Wm[1].T], 1).astype(f16)        # [256, 512]
    WaTnb = np.concatenate([Wa[0][:, D:].T, Wa[1][:, D:].T], 1).astype(f16)
    WaTcur = np.concatenate([Wa[0][:, :D].T, Wa[1][:, :D].T], 1).astype(f16)
    ba_col = np.concatenate([ba[0], ba[1]]).reshape(8, 1).astype(np.float32)
    bm_cat = np.concatenate([bm[0], bm[1]])                        # [512]
    biasr = (bih + bm_cat @ wih.T).reshape(1, -1).astype(f16)      # [1, 768]
    bhhr = bhh.reshape(1, -1).astype(f16)
    hmask = np.zeros((4, 4 * S), np.float32)
    for h in range(4):
        hmask[h, S * h:S * (h + 1)] = 1.0
    hmask = hmask.astype(f16)
    lnG = np.broadcast_to(ln_g, (128, D)).astype(np.float32).copy()
    lnB = np.broadcast_to(ln_b, (128, D)).astype(np.float32).copy()

    wc = [np.where(adj[e] != 0.0, w[e], -1.0).astype(f16) for e in range(2)]

    nc = _build_nc(dbg=_dbg)

    in_maps = []
    for c in range(NCORES):
        isl = slice(c * S, (c + 1) * S)
        m = {
            "wp0": _pack_tiles(wc[0][:, isl]),
            "wp1": _pack_tiles(wc[1][:, isl]),
            "xT": xT,
            "xisl": np.ascontiguousarray(xT[:, isl]),
            "wiT": wiT, "whT": whT, "WmT": WmT,
            "WaTnb": WaTnb, "WaTcur": WaTcur, "ba_col": ba_col,
            "biasr": biasr, "bhhr": bhhr, "hmask": hmask,
            "lnG": lnG, "lnB": lnB,
            "ones": np.ones((1, 128), f16),
            "ones2k": np.ones((1, N), f16),
            "onesf": np.ones((1, 64), np.float32),
            "ident": np.eye(128, dtype=f16),
            "onecol": np.ones((128, 128), f16),
            "zcol": np.zeros((1, 65), f16),
        }
        in_maps.append(m)

    import os
    trace = bool(int(os.environ.get("KERNEL_TRACE", "0")))
    if trace:
        try:
            import axon_ntff_shim  # noqa: F401  (registers the NTFF hook)
        except ImportError:
            trace = False
    res = run_bass_kernel_spmd(nc, in_maps, core_ids=list(range(NCORES)),
                               trace=trace)
    LAST_EXEC_NS = res.exec_time_ns
    out = np.concatenate([r["out"] for r in res.results], axis=0)
    if _dbg:
        global LAST_DBG
        LAST_DBG = res.results
    return out


# revision 5
# speedup vs baseline: 1.1340x; 1.0898x over previous
"""Trainium2 Bass kernel v2: GAT message passing (2 edge sets) + GRUCell + LayerNorm.

Key changes vs v1:
- Host pre-combines edge weight+mask: wc = w if adj else -1 (halves score DMA,
  removes all gpsimd adds, removes the fp16 saturation hack).
- Attention bias B'[j,(h,i)] = a_cur[i,h]+ba[h]+a_nb[j,h] built per j-tile by ONE
  k=5 PE matmul (ones/anbT rows x B_row/headmask), consumed directly from PSUM.
- ONE fused DVE score op per (set, j-tile) covering all 4 heads:
    u = select(wc < 0, -60000, leaky_relu(B' * wc, 0.2))
  (wc broadcast across heads via a stride-0 page dim). 32 calls instead of 128.
- exp batched per 2 j-tiles on ACT; msg PSUM->SBUF copies moved to gpsimd;
  softmax-normalize multiply moved to gpsimd; GRU bias row folded on host;
  rsqrt via ACT table switch instead of 5 Newton iterations on DVE.
- U matmuls emitted one 2-tile group late so PE never waits on ACT/DVE.
"""

import numpy as np

import concourse.bass as bass
import concourse.mybir as mybir
from concourse import bacc
import concourse.tile as tile
from concourse.bass_utils import run_bass_kernel_spmd

N, D, DH, H = 2048, 256, 256, 4
DHEAD = DH // H
NCORES = 8
S = N // NCORES          # 256 targets per core
JT = N // 128            # 16 j-tiles
KT = D // 128            # 2 k-tiles over d
F16 = mybir.dt.float16
F32 = mybir.dt.float32
AF = mybir.ActivationFunctionType
ALU = mybir.AluOpType

LAST_EXEC_NS = None

# ---------------------------------------------------------------- custom DVE op
_GATB_OP = None


def _register_gatb():
    """u = select(wc < 0, C0, leaky_relu(B' * wc, C1)).
    in0 = wc [P, 4(bcast), 256] fp16, in1 = B' [P, 4, 256] f32 (PSUM),
    s0 = mask value (-60000), s1 = leaky slope (0.2)."""
    global _GATB_OP
    if _GATB_OP is not None:
        return _GATB_OP
    import concourse.dve_ops as dve_ops
    from concourse.dve_spec import (
        C0, C1, Spec, Src0, Src1, Zero, _has_src1, lower as spec_lower,
        maxx, select,
    )
    from concourse.dve_uop import DveOpSpec

    name = "GATB_SCORE_ANT"
    for op in dve_ops.OPS:
        if op.name == name:
            _GATB_OP = op
            return op

    _q = Src0 * Src1
    body = select(Src0 < Zero, C0, maxx(_q, _q * C1))

    def _ref(in0, in1, s0, s1, imm2=None):
        q = in0.astype(np.float32) * in1.astype(np.float32)
        lr = np.maximum(q, q * np.float32(s1))
        return np.where(in0.astype(np.float32) < 0.0, np.float32(s0), lr).astype(
            np.float32
        )

    spec = Spec(body=body, reference=_ref)
    row = dve_ops._CUSTOM_DVE_ROW_BASE + len(dve_ops.OPS)
    shas = {}
    for ver in ("v3", "v4"):
        try:
            uops = spec_lower(spec, ver=ver)
            shas[ver] = DveOpSpec(
                name=name, opcode=row, uops=uops, rd1_en=_has_src1(spec)
            ).sha(ver)
        except Exception:
            pass
    op = dve_ops.DveOp(name, spec, subdim=False, uops_sha=shas,
                       perf_en={"v3": True, "v4": True})
    dve_ops.OPS.append(op)
    dve_ops.CUSTOM_DVE_SPECS[name] = spec
    dve_ops._SUB_OPCODE_FOR_NAME[name] = row
    _GATB_OP = op
    return op


# ---------------------------------------------------------------- bass program
_NC_CACHE = None


def _build_nc(dbg=False):
    global _NC_CACHE
    if _NC_CACHE is not None:
        return _NC_CACHE
    gatb = _register_gatb()

    nc = bacc.Bacc("TRN2", target_bir_lowering=False, debug=False,
                   enable_asserts=False)

    def din(nm, shape, dt):
        return nc.dram_tensor(nm, list(shape), dt, kind="ExternalInput").ap()

    wp = [din(f"wp{e}", (128, JT * S), F16) for e in range(2)]
    xT_d = din("xT", (D, N), F16)
    xisl_d = din("xisl", (D, S), F16)
    wiT_d = din("wiT", (8 * 65, 3 * D), F16)   # [520, 768] (zero row per piece)
    whT_d = din("whT", (D, 3 * D), F16)        # [256, 768]
    WmT_d = din("WmT", (D, 2 * DH), F16)       # [256, 512] (set0|set1 cols)
    WaTnb_d = din("WaTnb", (D, 8), F16)
    WaTcur_d = din("WaTcur", (D, 8), F16)
    ba_col_d = din("ba_col", (8, 1), F32)
    biasr_d = din("biasr", (1, 3 * D), F16)    # bih + bm_cat @ wih^T (host)
    bhhr_d = din("bhhr", (1, 3 * D), F16)
    hmask_d = din("hmask", (4, 4 * S), F16)    # hmask[h', h*S+i] = (h'==h)
    lnG_d = din("lnG", (128, D), F32)
    lnB_d = din("lnB", (128, D), F32)
    ones_d = din("ones", (1, 128), F16)
    ones2k_d = din("ones2k", (1, N), F16)
    onesf_d = din("onesf", (1, 64), F32)
    ident_d = din("ident", (128, 128), F16)
    onecol_d = din("onecol", (128, 128), F16)
    zcol_d = din("zcol", (1, 65), F16)

    out_d = nc.dram_tensor("out", [S, D], F32, kind="ExternalOutput").ap()
    dbg_d = {}
    if dbg:
        for nm, shape in [("d_Bp0", (128, 1024)), ("d_u0", (128, 1024)),
                          ("d_et0", (128, 1024)), ("d_anbT", (8, 2048)),
                          ("d_R0", (5, 1024)), ("d_aTcB", (8, 256)),
                          ("d_U00", (65, 256)), ("d_piece0", (64, 256)),
                          ("d_gh0", (128, 768)), ("d_gi0", (128, 768)),
                          ("d_hh0", (128, 256))]:
            dbg_d[nm] = nc.dram_tensor(nm, list(shape), F32,
                                       kind="ExternalOutput").ap()

    with tile.TileContext(nc) as tc:
        with (
            tc.tile_pool(name="const", bufs=1) as cp,
            tc.tile_pool(name="stream", bufs=1) as sp,
            tc.tile_pool(name="work", bufs=3) as wkp,
            tc.tile_pool(name="msg", bufs=1) as mp,
        ):
            def ddump(nm, ap):
                if not dbg or nm not in dbg_d:
                    return
                t = cp.tile(list(dbg_d[nm].shape), F32, tag=nm, name=nm)
                nc.vector.tensor_copy(t[:], ap)
                nc.sync.dma_start(out=dbg_d[nm][:, :], in_=t[:])

            def load(pool, nm, src, shape, dt, tag=None):
                t = pool.tile(shape, dt, tag=tag or nm, name=tag or nm)
                nc.sync.dma_start(out=t[:], in_=src)
                return t

            # ---------------- constants into SBUF (emission order = DMA order)
            xT = [load(cp, f"xT{k}", xT_d[128 * k:128 * (k + 1), :],
                       [128, N], F16) for k in range(KT)]
            xisl = [load(cp, f"xisl{k}", xisl_d[128 * k:128 * (k + 1), :],
                        [128, S], F16) for k in range(KT)]
            WaTnb = [load(cp, f"WaTnb{k}", WaTnb_d[128 * k:128 * (k + 1), :],
                          [128, 8], F16) for k in range(KT)]
            WaTcur = [load(cp, f"WaTcur{k}", WaTcur_d[128 * k:128 * (k + 1), :],
                           [128, 8], F16) for k in range(KT)]
            wsb = [sp.tile([128, JT * S], F16, tag=f"wsb{e}", name=f"wsb{e}")
                   for e in range(2)]
            nc.sync.dma_start(out=wsb[0][:], in_=wp[0][:, :])
            WmT = [load(cp, f"WmT{k}", WmT_d[128 * k:128 * (k + 1), :],
                        [128, 2 * DH], F16) for k in range(KT)]
            wiT = [load(cp, f"wiT{p}", wiT_d[65 * p:65 * (p + 1), :],
                        [65, 3 * D], F16) for p in range(8)]
            whT = [load(cp, f"whT{k}", whT_d[128 * k:128 * (k + 1), :],
                        [128, 3 * D], F16) for k in range(KT)]
            ba_col = load(cp, "ba_col", ba_col_d[:, :], [8, 1], F32)
            biasr = load(cp, "biasr", biasr_d[:, :], [1, 3 * D], F16)
            bhhr = load(cp, "bhhr", bhhr_d[:, :], [1, 3 * D], F16)
            lnG = load(cp, "lnG", lnG_d[:, :], [128, D], F32)
            lnB = load(cp, "lnB", lnB_d[:, :], [128, D], F32)
            ones = load(cp, "ones", ones_d[:, :], [1, 128], F16)
            zcol = load(cp, "zcol", zcol_d[:, :], [1, 65], F16)
            ones2k = load(cp, "ones2k", ones2k_d[:, :], [1, N], F16)
            onesf = load(cp, "onesf", onesf_d[:, :], [1, 64], F32)
            ident = load(cp, "ident", ident_d[:, :], [128, 128], F16)
            nc.sync.dma_start(out=wsb[1][:], in_=wp[1][:, :])

            # R_e = [B_row_e(filled later); headmask]  [5, 4S] fp16
            R = []
            for e in range(2):
                r = cp.tile([5, 4 * S], F16, tag=f"R{e}", name=f"R{e}")
                nc.gpsimd.dma_start(out=r[1:5, :], in_=hmask_d[:, :])
                R.append(r)
            # L_e = [ones_row; anbT_e(filled later)]  [5, N] fp16
            L = []
            for e in range(2):
                l_ = cp.tile([5, N], F16, tag=f"L{e}", name=f"L{e}")
                nc.gpsimd.dma_start(out=l_[0:1, :], in_=ones2k_d[:, :])
                L.append(l_)

            # me tiles, ones in column 64
            me_all = cp.tile([128, JT, 2, 4, 66], F16, tag="me", name="me")
            nc.gpsimd.memset(
                me_all[:, :, :, :, 0:1].rearrange("p a b c d -> p (a b c d)"),
                1.0)

            # ---------------- prep: anbT, aTcB, xn
            psP_cm = tc.tile_pool(name="psP", bufs=2, space="PSUM")
            psP = psP_cm.__enter__()

            anbT8 = cp.tile([8, N], F16, tag="anbT8", name="anbT8")
            for c in range(4):
                csl = slice(512 * c, 512 * (c + 1))
                ps = psP.tile([8, 512], F32, tag="ps_p", name="ps_anbT")
                for k in range(KT):
                    nc.tensor.matmul(ps[:], WaTnb[k][:], xT[k][:, csl],
                                     start=(k == 0), stop=(k == KT - 1))
                if c % 2 == 0:
                    nc.vector.tensor_copy(anbT8[:, csl], ps[:])
                else:
                    nc.scalar.copy(anbT8[:, csl], ps[:])
            ddump("d_anbT", anbT8[:, :])
            for e in range(2):
                nc.gpsimd.dma_start(out=L[e][1:5, :], in_=anbT8[4 * e:4 * e + 4, :])

            # a_cur^T + ba -> aTcB [8, S]; rows (4e+h) -> R_e row 0
            aTcB = cp.tile([8, S], F16, tag="aTcB", name="aTcB")
            for ih in range(2):
                ps = psP.tile([128, 8], F32, tag="ps_p", name="ps_ac")
                for k in range(KT):
                    nc.tensor.matmul(ps[:], xisl[k][:, 128 * ih:128 * (ih + 1)],
                                     WaTcur[k][:], start=(k == 0),
                                     stop=(k == KT - 1))
                ac = wkp.tile([128, 8], F16, tag="acur", name="acur")
                nc.vector.tensor_copy(ac[:], ps[:])
                pst = psP.tile([8, 128], F16, tag="ps_p2", name="ps_at")
                nc.tensor.transpose(pst[:], ac[:], ident[:])
                nc.vector.tensor_scalar_add(aTcB[:, 128 * ih:128 * (ih + 1)],
                                            pst[:], ba_col[:])
            ddump("d_aTcB", aTcB[:])
            for e in range(2):
                for h in range(4):
                    nc.gpsimd.dma_start(out=R[e][0:1, S * h:S * (h + 1)],
                                        in_=aTcB[4 * e + h:4 * e + h + 1, :])
            ddump("d_R0", R[0][:, :])

            # x islice natural layout [i, d] fp16 (for the GRU h-path)
            xn = []
            for ih in range(2):
                t = cp.tile([128, D], F16, tag=f"xn{ih}", name=f"xn{ih}")
                for k in range(KT):
                    pst = psP.tile([128, 128], F16, tag="ps_p2", name="ps_xt")
                    nc.tensor.transpose(
                        pst[:], xisl[k][:, 128 * ih:128 * (ih + 1)], ident[:])
                    nc.scalar.copy(t[:, 128 * k:128 * (k + 1)], pst[:])
                xn.append(t)
            # msg matmuls for ALL tiles, both sets (merged moving side)
            for t in range(JT):
                tsl = slice(128 * t, 128 * (t + 1))
                psm = psP.tile([128, 2, 4, DHEAD], F32, tag="ps_m",
                               name="ps_m", bufs=4)
                for k in range(KT):
                    nc.tensor.matmul(psm[:], xT[k][:, tsl], WmT[k][:],
                                     start=(k == 0), stop=(k == KT - 1))
                if t % 2 == 0:
                    nc.vector.tensor_copy(me_all[:, t, :, :, 1:65], psm[:])
                else:
                    nc.scalar.copy(me_all[:, t, :, :, 1:65], psm[:])
            psP_cm.__exit__(None, None, None)

            # ---------------- main: scores, exp, msg, aggregation
            psB_cm = tc.tile_pool(name="psB", bufs=3, space="PSUM")
            psU_cm = tc.tile_pool(name="psU", bufs=1, space="PSUM")
            psB = psB_cm.__enter__(); psU = psU_cm.__enter__()

            msgT = []
            for e in range(2):
                U = psU.tile([65, 4, S], F32, tag="ps_U", name="ps_U")
                for zb in range(2):
                    nc.tensor.matmul(
                        U[:, 2 * zb:2 * zb + 2, :].rearrange("p a b -> p (a b)"),
                        zcol[:], ones2k[0:1, 0:2 * S], start=True, stop=False,
                        skip_group_check=True)
                pend = []   # U-matmul groups not yet emitted
                for t in range(JT):
                    sl = slice(S * t, S * (t + 1))
                    tsl = slice(128 * t, 128 * (t + 1))
                    # B'[j,(h,i)] via one k=5 matmul
                    psb = psB.tile([128, 4, S], F32, tag="ps_B", name="ps_B")
                    for bh in range(2):
                        nc.tensor.matmul(
                            psb[:, 2 * bh:2 * bh + 2, :].rearrange(
                                "p a b -> p (a b)"),
                            L[e][:, tsl], R[e][:, 2 * S * bh:2 * S * (bh + 1)],
                            start=True, stop=True)
                    if e == 0 and t == 0:
                        ddump("d_Bp0", psb[:].rearrange("p a b -> p (a b)"))
                    # fused masked leaky score, all 4 heads in one call
                    if t % 2 == 0:
                        u2 = wkp.tile([128, 2, 4, S], F16, tag="u", name="u",
                                      bufs=3)
                    wc3 = (wsb[e][:, sl]
                           .rearrange("p (o n) -> p o n", o=1)
                           .broadcast_to([128, 4, S]))
                    nc.vector._custom_dve(
                        gatb,
                        out=u2[:, t % 2],
                        in0=wc3,
                        in1=psb[:],
                        s0=-60000.0,
                        s1=0.2,
                    )
                    if e == 0 and t == 0:
                        ddump("d_u0", u2[:, 0].rearrange("p a b -> p (a b)"))
                    if t % 2 == 1:
                        et2 = wkp.tile([128, 2, 4, S], F16, tag="et", name="et",
                                       bufs=3)
                        nc.scalar.activation(et2[:], u2[:], AF.Exp)
                        if e == 0 and t == 1:
                            ddump("d_et0", et2[:, 0].rearrange("p a b -> p (a b)"))
                        pend.append((t - 1, t, et2))
                        # emit the PREVIOUS group's U matmuls (keeps PE ahead)
                        if len(pend) == 2:
                            ta, tb, pet = pend.pop(0)
                            for tt in (ta, tb):
                                for h in range(4):
                                    nc.tensor.matmul(
                                        U[:, h, :], me_all[:, tt, e, h, 0:65],
                                        pet[:, tt % 2, h],
                                        start=False,
                                        stop=(tt == JT - 1 and h % 2 == 1),
                                        skip_group_check=True)
                for ta, tb, pet in pend:
                    for tt in (ta, tb):
                        for h in range(4):
                            nc.tensor.matmul(U[:, h, :], me_all[:, tt, e, h, 0:65],
                                             pet[:, tt % 2, h],
                                             start=False,
                                             stop=(tt == JT - 1 and h % 2 == 1),
                                             skip_group_check=True)

                if e == 0:
                    ddump("d_U00", U[:, 0, :])
                # normalize: piece = U[0:64] / U[64]
                for h in range(4):
                    rd = wkp.tile([1, S], F32, tag="rd", name="rd")
                    nc.vector.reciprocal_approx_fast(rd[0:1, :], U[0:1, h, :])
                    rb = wkp.tile([65, S], F32, tag="rb", name="rb", bufs=2)
                    nc.gpsimd.partition_broadcast(rb[:], rd[0:1, :])
                    piece = mp.tile([65, S], F16, tag=f"msgT{4 * e + h}",
                                    name=f"msgT{4 * e + h}")
                    nc.vector.tensor_tensor(piece[:], U[:, h, :], rb[:],
                                            ALU.mult)
                    if e == 0 and h == 0:
                        ddump("d_piece0", piece[1:65, :])
                    msgT.append(piece)

            # ---------------- GRU per i-half (psums borrow the psB buffers)
            hhs = []
            for ih in range(2):
                ihs = slice(128 * ih, 128 * (ih + 1))
                # gh = x @ whh^T + bhh
                psgh = psB.tile([128, 4, S], F32, tag="ps_B",
                                name="ps_gh")[:, :, :].rearrange(
                                    "p a b -> p (a b)")[:, 0:3 * D]
                for lo, hi in ((0, 512), (512, 768)):
                    for k in range(KT):
                        nc.tensor.matmul(psgh[:, lo:hi], xisl[k][:, ihs],
                                         whT[k][:, lo:hi], start=(k == 0),
                                         stop=False)
                    nc.tensor.matmul(psgh[:, lo:hi], ones[:], bhhr[:, lo:hi],
                                     start=False, stop=True)
                gh = wkp.tile([128, 3 * D], F32, tag="gh", name="gh", bufs=2)
                nc.scalar.copy(gh[:], psgh[:])
                if ih == 0:
                    ddump("d_gh0", gh[:])

                # gi = msgcat @ wih^T + (bih + bm@wihT)
                psgi = psB.tile([128, 4, S], F32, tag="ps_B",
                                name="ps_gi")[:, :, :].rearrange(
                                    "p a b -> p (a b)")[:, 0:3 * D]
                for lo, hi in ((0, 512), (512, 768)):
                    for p in range(8):
                        nc.tensor.matmul(psgi[:, lo:hi], msgT[p][:, ihs],
                                         wiT[p][:, lo:hi], start=(p == 0),
                                         stop=False)
                    nc.tensor.matmul(psgi[:, lo:hi], ones[:], biasr[:, lo:hi],
                                     start=False, stop=True)
                if ih == 0:
                    ddump("d_gi0", psgi[:, :])

                # r/z = sigmoid(gi+gh) = 0.5*tanh(0.5*(gi+gh)) + 0.5 ; n = tanh
                trz = wkp.tile([128, 2 * D], F32, tag="trz", name="trz", bufs=2)
                nc.vector.tensor_tensor(trz[:], psgi[:, 0:2 * D], gh[:, 0:2 * D],
                                        ALU.add)
                th = wkp.tile([128, 2 * D], F32, tag="th", name="th", bufs=2)
                nc.scalar.activation(th[:], trz[:], AF.Tanh, scale=0.5)
                rz = wkp.tile([128, 2 * D], F32, tag="rz", name="rz", bufs=2)
                nc.vector.tensor_scalar(rz[:], th[:], 0.5, 0.5, ALU.mult,
                                        ALU.add)
                t1 = wkp.tile([128, D], F32, tag="t1", name="t1", bufs=2)
                nc.gpsimd.tensor_mul(t1[:], rz[:, 0:D], gh[:, 2 * D:3 * D])
                t2 = wkp.tile([128, D], F32, tag="t2", name="t2", bufs=2)
                nc.vector.tensor_tensor(t2[:], t1[:], psgi[:, 2 * D:3 * D],
                                        ALU.add)
                nn_ = wkp.tile([128, D], F32, tag="nn", name="nn", bufs=2)
                nc.scalar.activation(nn_[:], t2[:], AF.Tanh)
                # h = n + z*(x - n)
                t3 = wkp.tile([128, D], F32, tag="t3", name="t3", bufs=2)
                nc.gpsimd.tensor_sub(t3[:], xn[ih][:], nn_[:])
                t4 = wkp.tile([128, D], F32, tag="t4", name="t4", bufs=2)
                nc.gpsimd.tensor_mul(t4[:], t3[:], rz[:, D:2 * D])
                hh = wkp.tile([128, D], F32, tag="hh", name="hh", bufs=2)
                nc.vector.tensor_tensor(hh[:], nn_[:], t4[:], ALU.add)
                if ih == 0:
                    ddump("d_hh0", hh[:])
                hhs.append(hh)

            # ---------------- LayerNorm per i-half (single sqrt table load)
            for ih in range(2):
                ihs = slice(128 * ih, 128 * (ih + 1))
                hh = hhs[ih]
                st = wkp.tile([128, 6], F32, tag="st", name="st", bufs=2)
                nc.vector.bn_stats(out=st[:], in_=hh[:])
                mv = wkp.tile([128, 2], F32, tag="mv", name="mv", bufs=2)
                nc.vector.bn_aggr(out=mv[:], in_=st[:])
                veps = wkp.tile([128, 1], F32, tag="veps", name="veps", bufs=2)
                nc.vector.tensor_scalar_add(veps[:], mv[:, 1:2], 1e-5)
                rcp = wkp.tile([128, 1], F32, tag="rcp", name="rcp", bufs=2)
                nc.vector.reciprocal(rcp[:], veps[:])
                rv = wkp.tile([128, 1], F32, tag="rv", name="rv", bufs=2)
                nc.scalar.activation(rv[:], rcp[:], AF.Sqrt)
                hn = wkp.tile([128, D], F32, tag="hn", name="hn", bufs=2)
                nc.vector.tensor_scalar(hn[:], hh[:], mv[:, 0:1], rv[:],
                                        ALU.subtract, ALU.mult)
                og = wkp.tile([128, D], F32, tag="og", name="og", bufs=2)
                nc.gpsimd.tensor_mul(og[:], hn[:], lnG[:])
                ob = wkp.tile([128, D], F32, tag="ob", name="ob", bufs=2)
                nc.vector.tensor_tensor(ob[:], og[:], lnB[:], ALU.add)
                nc.sync.dma_start(out=out_d[ihs, :], in_=ob[:])
            psU_cm.__exit__(None, None, None)
            psB_cm.__exit__(None, None, None)

    nc.compile()
    _NC_CACHE = nc
    return nc


# ---------------------------------------------------------------- host wrapper
def _pack_tiles(a):
    """[N, S] -> [128, JT*S]: row-tile t, partition p holds a[t*128+p, :] at
    cols [t*S:(t+1)*S]."""
    n, s = a.shape
    t = n // 128
    return np.ascontiguousarray(
        a.reshape(t, 128, s).transpose(1, 0, 2).reshape(128, t * s))


def kernel(_dbg=False, **inputs):
    global LAST_EXEC_NS
    f16 = np.float16
    x = np.asarray(inputs["axiom_states"], np.float32)
    adj = [np.asarray(inputs["adj0"], np.float32),
           np.asarray(inputs["adj1"], np.float32)]
    w = [np.asarray(inputs["w0"], np.float32),
         np.asarray(inputs["w1"], np.float32)]
    Wm = [np.asarray(inputs["Wm0"], np.float32),
          np.asarray(inputs["Wm1"], np.float32)]
    bm = [np.asarray(inputs["bm0"], np.float32),
          np.asarray(inputs["bm1"], np.float32)]
    Wa = [np.asarray(inputs["Wa0"], np.float32),
          np.asarray(inputs["Wa1"], np.float32)]
    ba = [np.asarray(inputs["ba0"], np.float32),
          np.asarray(inputs["ba1"], np.float32)]
    wih = np.asarray(inputs["gru_wih"], np.float32)
    whh = np.asarray(inputs["gru_whh"], np.float32)
    bih = np.asarray(inputs["gru_bih"], np.float32)
    bhh = np.asarray(inputs["gru_bhh"], np.float32)
    ln_g = np.asarray(inputs["ln_g"], np.float32)
    ln_b = np.asarray(inputs["ln_b"], np.float32)

    xT = np.ascontiguousarray(x.T).astype(f16)                     # [256, 2048]
    wiTraw = np.ascontiguousarray(wih.T).astype(np.float32)        # [512, 768]
    wiT = np.zeros((8 * 65, 768), np.float32)
    for p in range(8):
        wiT[65 * p + 1:65 * (p + 1)] = wiTraw[64 * p:64 * (p + 1)]
    wiT = wiT.astype(f16)
    whT = np.ascontiguousarray(whh.T).astype(f16)                  # [256, 768]
    WmT = np.concatenate([Wm[0].T, Wm[1].T], 1).astype(f16)        # [256, 512]
    WaTnb = np.concatenate([Wa[0][:, D:].T, Wa[1][:, D:].T], 1).astype(f16)
    WaTcur = np.concatenate([Wa[0][:, :D].T, Wa[1][:, :D].T], 1).astype(f16)
    ba_col = np.concatenate([ba[0], ba[1]]).reshape(8, 1).astype(np.float32)
    bm_cat = np.concatenate([bm[0], bm[1]])                        # [512]
    biasr = (bih + bm_cat @ wih.T).reshape(1, -1).astype(f16)      # [1, 768]
    bhhr = bhh.reshape(1, -1).astype(f16)
    hmask = np.zeros((4, 4 * S), np.float32)
    for h in range(4):
        hmask[h, S * h:S * (h + 1)] = 1.0
    hmask = hmask.astype(f16)
    lnG = np.broadcast_to(ln_g, (128, D)).astype(np.float32).copy()
    lnB = np.broadcast_to(ln_b, (128, D)).astype(np.float32).copy()

    wc = [np.where(adj[e] != 0.0, w[e], -1.0).astype(f16) for e in range(2)]

    nc = _build_nc(dbg=_dbg)

    in_maps = []
    for c in range(NCORES):
        isl = slice(c * S, (c + 1) * S)
        m = {
            "wp0": _pack_tiles(wc[0][:, isl]),
            "wp1": _pack_tiles(wc[1][:, isl]),
            "xT": xT,
            "xisl": np.ascontiguousarray(xT[:, isl]),
            "wiT": wiT, "whT": whT, "WmT": WmT,
            "WaTnb": WaTnb, "WaTcur": WaTcur, "ba_col": ba_col,
            "biasr": biasr, "bhhr": bhhr, "hmask": hmask,
            "lnG": lnG, "lnB": lnB,
            "ones": np.ones((1, 128), f16),
            "ones2k": np.ones((1, N), f16),
            "onesf": np.ones((1, 64), np.float32),
            "ident": np.eye(128, dtype=f16),
            "onecol": np.ones((128, 128), f16),
            "zcol": np.zeros((1, 65), f16),
        }
        in_maps.append(m)

    import os
    trace = bool(int(os.environ.get("KERNEL_TRACE", "0")))
    if trace:
        try:
            import axon_ntff_shim  # noqa: F401  (registers the NTFF hook)
        except ImportError:
            trace = False
    res = run_bass_kernel_spmd(nc, in_maps, core_ids=list(range(NCORES)),
                               trace=trace)
    LAST_EXEC_NS = res.exec_time_ns
    out = np.concatenate([r["out"] for r in res.results], axis=0)
    if _dbg:
        global LAST_DBG
        LAST_DBG = res.results
    return out


# revision 6
# speedup vs baseline: 1.1700x; 1.0318x over previous
"""Trainium2 Bass kernel v2: GAT message passing (2 edge sets) + GRUCell + LayerNorm.

Key changes vs v1:
- Host pre-combines edge weight+mask: wc = w if adj else -1 (halves score DMA,
  removes all gpsimd adds, removes the fp16 saturation hack).
- Attention bias B'[j,(h,i)] = a_cur[i,h]+ba[h]+a_nb[j,h] built per j-tile by ONE
  k=5 PE matmul (ones/anbT rows x B_row/headmask), consumed directly from PSUM.
- ONE fused DVE score op per (set, j-tile) covering all 4 heads:
    u = select(wc < 0, -60000, leaky_relu(B' * wc, 0.2))
  (wc broadcast across heads via a stride-0 page dim). 32 calls instead of 128.
- exp batched per 2 j-tiles on ACT; msg PSUM->SBUF copies moved to gpsimd;
  softmax-normalize multiply moved to gpsimd; GRU bias row folded on host;
  rsqrt via ACT table switch instead of 5 Newton iterations on DVE.
- U matmuls emitted one 2-tile group late so PE never waits on ACT/DVE.
"""

import numpy as np

import concourse.bass as bass
import concourse.mybir as mybir
from concourse import bacc
import concourse.tile as tile
from concourse.bass_utils import run_bass_kernel_spmd

N, D, DH, H = 2048, 256, 256, 4
DHEAD = DH // H
NCORES = 8
S = N // NCORES          # 256 targets per core
JT = N // 128            # 16 j-tiles
KT = D // 128            # 2 k-tiles over d
F16 = mybir.dt.float16
F32 = mybir.dt.float32
AF = mybir.ActivationFunctionType
ALU = mybir.AluOpType

LAST_EXEC_NS = None

# ---------------------------------------------------------------- custom DVE op
_GATB_OP = None


def _register_gatb():
    """u = select(wc < 0, C0, leaky_relu(B' * wc, C1)).
    in0 = wc [P, 4(bcast), 256] fp16, in1 = B' [P, 4, 256] f32 (PSUM),
    s0 = mask value (-60000), s1 = leaky slope (0.2)."""
    global _GATB_OP
    if _GATB_OP is not None:
        return _GATB_OP
    import concourse.dve_ops as dve_ops
    from concourse.dve_spec import (
        C0, C1, Spec, Src0, Src1, Zero, _has_src1, lower as spec_lower,
        maxx, select,
    )
    from concourse.dve_uop import DveOpSpec

    name = "GATB_SCORE_ANT"
    for op in dve_ops.OPS:
        if op.name == name:
            _GATB_OP = op
            return op

    _q = Src0 * Src1
    body = select(Src0 < Zero, C0, maxx(_q, _q * C1))

    def _ref(in0, in1, s0, s1, imm2=None):
        q = in0.astype(np.float32) * in1.astype(np.float32)
        lr = np.maximum(q, q * np.float32(s1))
        return np.where(in0.astype(np.float32) < 0.0, np.float32(s0), lr).astype(
            np.float32
        )

    spec = Spec(body=body, reference=_ref)
    row = dve_ops._CUSTOM_DVE_ROW_BASE + len(dve_ops.OPS)
    shas = {}
    for ver in ("v3", "v4"):
        try:
            uops = spec_lower(spec, ver=ver)
            shas[ver] = DveOpSpec(
                name=name, opcode=row, uops=uops, rd1_en=_has_src1(spec)
            ).sha(ver)
        except Exception:
            pass
    op = dve_ops.DveOp(name, spec, subdim=False, uops_sha=shas,
                       perf_en={"v3": True, "v4": True})
    dve_ops.OPS.append(op)
    dve_ops.CUSTOM_DVE_SPECS[name] = spec
    dve_ops._SUB_OPCODE_FOR_NAME[name] = row
    _GATB_OP = op
    return op


# ---------------------------------------------------------------- bass program
_NC_CACHE = None


def _build_nc(dbg=False):
    global _NC_CACHE
    if _NC_CACHE is not None:
        return _NC_CACHE
    gatb = _register_gatb()

    nc = bacc.Bacc("TRN2", target_bir_lowering=False, debug=False,
                   enable_asserts=False)

    def din(nm, shape, dt):
        return nc.dram_tensor(nm, list(shape), dt, kind="ExternalInput").ap()

    wp = [din(f"wp{e}", (128, JT * S), F16) for e in range(2)]
    xT_d = din("xT", (D, N), F16)
    xisl_d = din("xisl", (D, S), F16)
    wiT_d = din("wiT", (8 * 65, 3 * D), F16)   # [520, 768] (zero row per piece)
    whT_d = din("whT", (D, 3 * D), F16)        # [256, 768]
    WmT_d = din("WmT", (D, 2 * DH), F16)       # [256, 512] (set0|set1 cols)
    WaTnb_d = din("WaTnb", (D, 8), F16)
    WaTcur_d = din("WaTcur", (D, 8), F16)
    ba_col_d = din("ba_col", (8, 1), F32)
    biasr_d = din("biasr", (1, 3 * D), F16)    # bih + bm_cat @ wih^T (host)
    bhhr_d = din("bhhr", (1, 3 * D), F16)
    hmask_d = din("hmask", (4, 4 * S), F16)    # hmask[h', h*S+i] = (h'==h)
    lnG_d = din("lnG", (128, D), F32)
    lnB_d = din("lnB", (128, D), F32)
    ones_d = din("ones", (1, 128), F16)
    ones2k_d = din("ones2k", (1, N), F16)
    onesf_d = din("onesf", (1, 64), F32)
    ident_d = din("ident", (128, 128), F16)
    onecol_d = din("onecol", (128, 128), F16)
    zcol_d = din("zcol", (1, 65), F16)

    out_d = nc.dram_tensor("out", [S, D], F32, kind="ExternalOutput").ap()
    dbg_d = {}
    if dbg:
        for nm, shape in [("d_Bp0", (128, 1024)), ("d_u0", (128, 1024)),
                          ("d_et0", (128, 1024)), ("d_anbT", (8, 2048)),
                          ("d_R0", (5, 1024)), ("d_aTcB", (8, 256)),
                          ("d_U00", (65, 256)), ("d_piece0", (64, 256)),
                          ("d_gh0", (128, 768)), ("d_gi0", (128, 768)),
                          ("d_hh0", (128, 256))]:
            dbg_d[nm] = nc.dram_tensor(nm, list(shape), F32,
                                       kind="ExternalOutput").ap()

    with tile.TileContext(nc) as tc:
        with (
            tc.tile_pool(name="const", bufs=1) as cp,
            tc.tile_pool(name="stream", bufs=1) as sp,
            tc.tile_pool(name="work", bufs=3) as wkp,
            tc.tile_pool(name="msg", bufs=1) as mp,
        ):
            def ddump(nm, ap):
                if not dbg or nm not in dbg_d:
                    return
                t = cp.tile(list(dbg_d[nm].shape), F32, tag=nm, name=nm)
                nc.vector.tensor_copy(t[:], ap)
                nc.sync.dma_start(out=dbg_d[nm][:, :], in_=t[:])

            def load(pool, nm, src, shape, dt, tag=None):
                t = pool.tile(shape, dt, tag=tag or nm, name=tag or nm)
                nc.sync.dma_start(out=t[:], in_=src)
                return t

            # ---------------- constants into SBUF (emission order = DMA order)
            xT = [load(cp, f"xT{k}", xT_d[128 * k:128 * (k + 1), :],
                       [128, N], F16) for k in range(KT)]
            xisl = [load(cp, f"xisl{k}", xisl_d[128 * k:128 * (k + 1), :],
                        [128, S], F16) for k in range(KT)]
            WaTnb = [load(cp, f"WaTnb{k}", WaTnb_d[128 * k:128 * (k + 1), :],
                          [128, 8], F16) for k in range(KT)]
            WaTcur = [load(cp, f"WaTcur{k}", WaTcur_d[128 * k:128 * (k + 1), :],
                           [128, 8], F16) for k in range(KT)]
            wsb = [sp.tile([128, JT * S], F16, tag=f"wsb{e}", name=f"wsb{e}")
                   for e in range(2)]
            nc.sync.dma_start(out=wsb[0][:], in_=wp[0][:, :])
            WmT = [load(cp, f"WmT{k}", WmT_d[128 * k:128 * (k + 1), :],
                        [128, 2 * DH], F16) for k in range(KT)]
            wiT = [load(cp, f"wiT{p}", wiT_d[65 * p:65 * (p + 1), :],
                        [65, 3 * D], F16) for p in range(8)]
            whT = [load(cp, f"whT{k}", whT_d[128 * k:128 * (k + 1), :],
                        [128, 3 * D], F16) for k in range(KT)]
            ba_col = load(cp, "ba_col", ba_col_d[:, :], [8, 1], F32)
            biasr = load(cp, "biasr", biasr_d[:, :], [1, 3 * D], F16)
            bhhr = load(cp, "bhhr", bhhr_d[:, :], [1, 3 * D], F16)
            lnG = load(cp, "lnG", lnG_d[:, :], [128, D], F32)
            lnB = load(cp, "lnB", lnB_d[:, :], [128, D], F32)
            ones = load(cp, "ones", ones_d[:, :], [1, 128], F16)
            zcol = load(cp, "zcol", zcol_d[:, :], [1, 65], F16)
            ones2k = load(cp, "ones2k", ones2k_d[:, :], [1, N], F16)
            onesf = load(cp, "onesf", onesf_d[:, :], [1, 64], F32)
            ident = load(cp, "ident", ident_d[:, :], [128, 128], F16)
            nc.sync.dma_start(out=wsb[1][:], in_=wp[1][:, :])

            # R_e = [B_row_e(filled later); headmask]  [5, 4S] fp16
            R = []
            for e in range(2):
                r = cp.tile([5, 4 * S], F16, tag=f"R{e}", name=f"R{e}")
                nc.gpsimd.dma_start(out=r[1:5, :], in_=hmask_d[:, :])
                R.append(r)
            # L_e = [ones_row; anbT_e(filled later)]  [5, N] fp16
            L = []
            for e in range(2):
                l_ = cp.tile([5, N], F16, tag=f"L{e}", name=f"L{e}")
                nc.gpsimd.dma_start(out=l_[0:1, :], in_=ones2k_d[:, :])
                L.append(l_)

            # me tiles, ones in column 64
            me_q = [cp.tile([128, 4, 2, 4, 66], F16, tag=f"me{q}",
                            name=f"me{q}") for q in range(4)]
            for q in range(4):
                nc.gpsimd.memset(
                    me_q[q][:, :, :, :, 0:1].rearrange("p a b c d -> p (a b c d)"),
                    1.0)

            # ---------------- prep: anbT, aTcB, xn
            psP_cm = tc.tile_pool(name="psP", bufs=2, space="PSUM")
            psP = psP_cm.__enter__()

            anbT8 = cp.tile([8, N], F16, tag="anbT8", name="anbT8")
            for c in range(4):
                csl = slice(512 * c, 512 * (c + 1))
                ps = psP.tile([8, 512], F32, tag="ps_p", name="ps_anbT")
                for k in range(KT):
                    nc.tensor.matmul(ps[:], WaTnb[k][:], xT[k][:, csl],
                                     start=(k == 0), stop=(k == KT - 1))
                if c % 2 == 0:
                    nc.vector.tensor_copy(anbT8[:, csl], ps[:])
                else:
                    nc.scalar.copy(anbT8[:, csl], ps[:])
            ddump("d_anbT", anbT8[:, :])
            for e in range(2):
                nc.gpsimd.dma_start(out=L[e][1:5, :], in_=anbT8[4 * e:4 * e + 4, :])

            # a_cur^T + ba -> aTcB [8, S]; rows (4e+h) -> R_e row 0
            aTcB = cp.tile([8, S], F16, tag="aTcB", name="aTcB")
            for ih in range(2):
                ps = psP.tile([128, 8], F32, tag="ps_p", name="ps_ac")
                for k in range(KT):
                    nc.tensor.matmul(ps[:], xisl[k][:, 128 * ih:128 * (ih + 1)],
                                     WaTcur[k][:], start=(k == 0),
                                     stop=(k == KT - 1))
                ac = wkp.tile([128, 8], F16, tag="acur", name="acur")
                nc.vector.tensor_copy(ac[:], ps[:])
                pst = psP.tile([8, 128], F16, tag="ps_p2", name="ps_at")
                nc.tensor.transpose(pst[:], ac[:], ident[:])
                nc.vector.tensor_scalar_add(aTcB[:, 128 * ih:128 * (ih + 1)],
                                            pst[:], ba_col[:])
            ddump("d_aTcB", aTcB[:])
            for e in range(2):
                for h in range(4):
                    nc.gpsimd.dma_start(out=R[e][0:1, S * h:S * (h + 1)],
                                        in_=aTcB[4 * e + h:4 * e + h + 1, :])
            ddump("d_R0", R[0][:, :])

            # x islice natural layout [i, d] fp16 (for the GRU h-path)
            xn = []
            for ih in range(2):
                t = cp.tile([128, D], F16, tag=f"xn{ih}", name=f"xn{ih}")
                for k in range(KT):
                    pst = psP.tile([128, 128], F16, tag="ps_p2", name="ps_xt")
                    nc.tensor.transpose(
                        pst[:], xisl[k][:, 128 * ih:128 * (ih + 1)], ident[:])
                    nc.scalar.copy(t[:, 128 * k:128 * (k + 1)], pst[:])
                xn.append(t)
            # msg matmuls for ALL tiles, both sets (merged moving side)
            for t in range(JT):
                tsl = slice(128 * t, 128 * (t + 1))
                psm = psP.tile([128, 2, 4, DHEAD], F32, tag="ps_m",
                               name="ps_m", bufs=4)
                for k in range(KT):
                    nc.tensor.matmul(psm[:], xT[k][:, tsl], WmT[k][:],
                                     start=(k == 0), stop=(k == KT - 1))
                if t % 2 == 0:
                    nc.vector.tensor_copy(me_q[t // 4][:, t % 4, :, :, 1:65],
                                          psm[:])
                else:
                    nc.scalar.copy(me_q[t // 4][:, t % 4, :, :, 1:65], psm[:])
            psP_cm.__exit__(None, None, None)

            # ---------------- main: scores, exp, msg, aggregation
            psB_cm = tc.tile_pool(name="psB", bufs=3, space="PSUM")
            psU_cm = tc.tile_pool(name="psU", bufs=1, space="PSUM")
            psB = psB_cm.__enter__(); psU = psU_cm.__enter__()

            msgT = []
            for e in range(2):
                U = psU.tile([65, 4, S], F32, tag="ps_U", name="ps_U")
                for zb in range(2):
                    nc.tensor.matmul(
                        U[:, 2 * zb:2 * zb + 2, :].rearrange("p a b -> p (a b)"),
                        zcol[:], ones2k[0:1, 0:2 * S], start=True, stop=False,
                        skip_group_check=True)
                pend = []   # U-matmul groups not yet emitted
                for t in range(JT):
                    sl = slice(S * t, S * (t + 1))
                    tsl = slice(128 * t, 128 * (t + 1))
                    # B'[j,(h,i)] via one k=5 matmul
                    psb = psB.tile([128, 4, S], F32, tag="ps_B", name="ps_B")
                    for bh in range(2):
                        nc.tensor.matmul(
                            psb[:, 2 * bh:2 * bh + 2, :].rearrange(
                                "p a b -> p (a b)"),
                            L[e][:, tsl], R[e][:, 2 * S * bh:2 * S * (bh + 1)],
                            start=True, stop=True)
                    if e == 0 and t == 0:
                        ddump("d_Bp0", psb[:].rearrange("p a b -> p (a b)"))
                    # fused masked leaky score, all 4 heads in one call
                    if t % 2 == 0:
                        u2 = wkp.tile([128, 2, 4, S], F16, tag="u", name="u",
                                      bufs=3)
                    wc3 = (wsb[e][:, sl]
                           .rearrange("p (o n) -> p o n", o=1)
                           .broadcast_to([128, 4, S]))
                    nc.vector._custom_dve(
                        gatb,
                        out=u2[:, t % 2],
                        in0=wc3,
                        in1=psb[:],
                        s0=-60000.0,
                        s1=0.2,
                    )
                    if e == 0 and t == 0:
                        ddump("d_u0", u2[:, 0].rearrange("p a b -> p (a b)"))
                    if t % 2 == 1:
                        et2 = wkp.tile([128, 2, 4, S], F16, tag="et", name="et",
                                       bufs=3)
                        nc.scalar.activation(et2[:], u2[:], AF.Exp)
                        if e == 0 and t == 1:
                            ddump("d_et0", et2[:, 0].rearrange("p a b -> p (a b)"))
                        pend.append((t - 1, t, et2))
                        # emit the PREVIOUS group's U matmuls (keeps PE ahead)
                        if len(pend) == 2:
                            ta, tb, pet = pend.pop(0)
                            for tt in (ta, tb):
                                for h in range(4):
                                    nc.tensor.matmul(
                                        U[:, h, :],
                                        me_q[tt // 4][:, tt % 4, e, h, 0:65],
                                        pet[:, tt % 2, h],
                                        start=False,
                                        stop=(tt == JT - 1 and h % 2 == 1),
                                        skip_group_check=True)
                for ta, tb, pet in pend:
                    for tt in (ta, tb):
                        for h in range(4):
                            nc.tensor.matmul(
                                U[:, h, :], me_q[tt // 4][:, tt % 4, e, h, 0:65],
                                             pet[:, tt % 2, h],
                                             start=False,
                                             stop=(tt == JT - 1 and h % 2 == 1),
                                             skip_group_check=True)

                if e == 0:
                    ddump("d_U00", U[:, 0, :])
                # normalize: piece = U[0:64] / U[64]
                for h in range(4):
                    rd = wkp.tile([1, S], F32, tag="rd", name="rd")
                    nc.vector.reciprocal_approx_fast(rd[0:1, :], U[0:1, h, :])
                    rb = wkp.tile([65, S], F32, tag="rb", name="rb", bufs=2)
                    nc.gpsimd.partition_broadcast(rb[:], rd[0:1, :])
                    piece = mp.tile([65, S], F16, tag=f"msgT{4 * e + h}",
                                    name=f"msgT{4 * e + h}")
                    nc.vector.tensor_tensor(piece[:], U[:, h, :], rb[:],
                                            ALU.mult)
                    if e == 0 and h == 0:
                        ddump("d_piece0", piece[1:65, :])
                    msgT.append(piece)

            # ---------------- GRU per i-half (psums borrow the psB buffers)
            hhs = []
            for ih in range(2):
                ihs = slice(128 * ih, 128 * (ih + 1))
                # gh = x @ whh^T + bhh
                psgh = psB.tile([128, 4, S], F32, tag="ps_B",
                                name="ps_gh")[:, :, :].rearrange(
                                    "p a b -> p (a b)")[:, 0:3 * D]
                for lo, hi in ((0, 512), (512, 768)):
                    for k in range(KT):
                        nc.tensor.matmul(psgh[:, lo:hi], xisl[k][:, ihs],
                                         whT[k][:, lo:hi], start=(k == 0),
                                         stop=False)
                    nc.tensor.matmul(psgh[:, lo:hi], ones[:], bhhr[:, lo:hi],
                                     start=False, stop=True)
                gh = wkp.tile([128, 3 * D], F32, tag="gh", name="gh", bufs=2)
                nc.scalar.copy(gh[:], psgh[:])
                if ih == 0:
                    ddump("d_gh0", gh[:])

                # gi = msgcat @ wih^T + (bih + bm@wihT)
                psgi = psB.tile([128, 4, S], F32, tag="ps_B",
                                name="ps_gi")[:, :, :].rearrange(
                                    "p a b -> p (a b)")[:, 0:3 * D]
                for lo, hi in ((0, 512), (512, 768)):
                    for p in range(8):
                        nc.tensor.matmul(psgi[:, lo:hi], msgT[p][:, ihs],
                                         wiT[p][:, lo:hi], start=(p == 0),
                                         stop=False)
                    nc.tensor.matmul(psgi[:, lo:hi], ones[:], biasr[:, lo:hi],
                                     start=False, stop=True)
                if ih == 0:
                    ddump("d_gi0", psgi[:, :])

                # r/z = sigmoid(gi+gh) = 0.5*tanh(0.5*(gi+gh)) + 0.5 ; n = tanh
                trz = wkp.tile([128, 2 * D], F32, tag="trz", name="trz", bufs=2)
                nc.vector.tensor_tensor(trz[:], psgi[:, 0:2 * D], gh[:, 0:2 * D],
                                        ALU.add)
                th = wkp.tile([128, 2 * D], F32, tag="th", name="th", bufs=2)
                nc.scalar.activation(th[:], trz[:], AF.Tanh, scale=0.5)
                rz = wkp.tile([128, 2 * D], F32, tag="rz", name="rz", bufs=2)
                nc.vector.tensor_scalar(rz[:], th[:], 0.5, 0.5, ALU.mult,
                                        ALU.add)
                t1 = wkp.tile([128, D], F32, tag="t1", name="t1", bufs=2)
                nc.vector.tensor_tensor(t1[:], rz[:, 0:D], gh[:, 2 * D:3 * D],
                                        ALU.mult)
                t2 = wkp.tile([128, D], F32, tag="t2", name="t2", bufs=2)
                nc.vector.tensor_tensor(t2[:], t1[:], psgi[:, 2 * D:3 * D],
                                        ALU.add)
                nn_ = wkp.tile([128, D], F32, tag="nn", name="nn", bufs=2)
                nc.scalar.activation(nn_[:], t2[:], AF.Tanh)
                # h = n + z*(x - n)
                t3 = wkp.tile([128, D], F32, tag="t3", name="t3", bufs=2)
                nc.vector.tensor_tensor(t3[:], xn[ih][:], nn_[:],
                                        ALU.subtract)
                t4 = wkp.tile([128, D], F32, tag="t4", name="t4", bufs=2)
                nc.vector.tensor_tensor(t4[:], t3[:], rz[:, D:2 * D],
                                        ALU.mult)
                hh = wkp.tile([128, D], F32, tag="hh", name="hh", bufs=2)
                nc.vector.tensor_tensor(hh[:], nn_[:], t4[:], ALU.add)
                if ih == 0:
                    ddump("d_hh0", hh[:])
                hhs.append(hh)

            # ---------------- LayerNorm per i-half (single sqrt table load)
            for ih in range(2):
                ihs = slice(128 * ih, 128 * (ih + 1))
                hh = hhs[ih]
                st = wkp.tile([128, 6], F32, tag="st", name="st", bufs=2)
                nc.vector.bn_stats(out=st[:], in_=hh[:])
                mv = wkp.tile([128, 2], F32, tag="mv", name="mv", bufs=2)
                nc.vector.bn_aggr(out=mv[:], in_=st[:])
                veps = wkp.tile([128, 1], F32, tag="veps", name="veps", bufs=2)
                nc.vector.tensor_scalar_add(veps[:], mv[:, 1:2], 1e-5)
                rcp = wkp.tile([128, 1], F32, tag="rcp", name="rcp", bufs=2)
                nc.vector.reciprocal(rcp[:], veps[:])
                rv = wkp.tile([128, 1], F32, tag="rv", name="rv", bufs=2)
                nc.scalar.activation(rv[:], rcp[:], AF.Sqrt)
                hn = wkp.tile([128, D], F32, tag="hn", name="hn", bufs=2)
                nc.vector.tensor_scalar(hn[:], hh[:], mv[:, 0:1], rv[:],
                                        ALU.subtract, ALU.mult)
                og = wkp.tile([128, D], F32, tag="og", name="og", bufs=2)
                nc.vector.tensor_tensor(og[:], hn[:], lnG[:], ALU.mult)
                ob = wkp.tile([128, D], F32, tag="ob", name="ob", bufs=2)
                nc.vector.tensor_tensor(ob[:], og[:], lnB[:], ALU.add)
                nc.sync.dma_start(out=out_d[ihs, :], in_=ob[:])
            psU_cm.__exit__(None, None, None)
            psB_cm.__exit__(None, None, None)

    nc.compile()
    _NC_CACHE = nc
    return nc


# ---------------------------------------------------------------- host wrapper
def _pack_tiles(a):
    """[N, S] -> [128, JT*S]: row-tile t, partition p holds a[t*128+p, :] at
    cols [t*S:(t+1)*S]."""
    n, s = a.shape
    t = n // 128
    return np.ascontiguousarray(
        a.reshape(t, 128, s).transpose(1, 0, 2).reshape(128, t * s))


def kernel(_dbg=False, **inputs):
    global LAST_EXEC_NS
    f16 = np.float16
    x = np.asarray(inputs["axiom_states"], np.float32)
    adj = [np.asarray(inputs["adj0"], np.float32),
           np.asarray(inputs["adj1"], np.float32)]
    w = [np.asarray(inputs["w0"], np.float32),
         np.asarray(inputs["w1"], np.float32)]
    Wm = [np.asarray(inputs["Wm0"], np.float32),
          np.asarray(inputs["Wm1"], np.float32)]
    bm = [np.asarray(inputs["bm0"], np.float32),
          np.asarray(inputs["bm1"], np.float32)]
    Wa = [np.asarray(inputs["Wa0"], np.float32),
          np.asarray(inputs["Wa1"], np.float32)]
    ba = [np.asarray(inputs["ba0"], np.float32),
          np.asarray(inputs["ba1"], np.float32)]
    wih = np.asarray(inputs["gru_wih"], np.float32)
    whh = np.asarray(inputs["gru_whh"], np.float32)
    bih = np.asarray(inputs["gru_bih"], np.float32)
    bhh = np.asarray(inputs["gru_bhh"], np.float32)
    ln_g = np.asarray(inputs["ln_g"], np.float32)
    ln_b = np.asarray(inputs["ln_b"], np.float32)

    xT = np.ascontiguousarray(x.T).astype(f16)                     # [256, 2048]
    wiTraw = np.ascontiguousarray(wih.T).astype(np.float32)        # [512, 768]
    wiT = np.zeros((8 * 65, 768), np.float32)
    for p in range(8):
        wiT[65 * p + 1:65 * (p + 1)] = wiTraw[64 * p:64 * (p + 1)]
    wiT = wiT.astype(f16)
    whT = np.ascontiguousarray(whh.T).astype(f16)                  # [256, 768]
    WmT = np.concatenate([Wm[0].T, Wm[1].T], 1).astype(f16)        # [256, 512]
    WaTnb = np.concatenate([Wa[0][:, D:].T, Wa[1][:, D:].T], 1).astype(f16)
    WaTcur = np.concatenate([Wa[0][:, :D].T, Wa[1][:, :D].T], 1).astype(f16)
    ba_col = np.concatenate([ba[0], ba[1]]).reshape(8, 1).astype(np.float32)
    bm_cat = np.concatenate([bm[0], bm[1]])                        # [512]
    biasr = (bih + bm_cat @ wih.T).reshape(1, -1).astype(f16)      # [1, 768]
    bhhr = bhh.reshape(1, -1).astype(f16)
    hmask = np.zeros((4, 4 * S), np.float32)
    for h in range(4):
        hmask[h, S * h:S * (h + 1)] = 1.0
    hmask = hmask.astype(f16)
    lnG = np.broadcast_to(ln_g, (128, D)).astype(np.float32).copy()
    lnB = np.broadcast_to(ln_b, (128, D)).astype(np.float32).copy()

    wc = [np.where(adj[e] != 0.0, w[e], -1.0).astype(f16) for e in range(2)]

    nc = _build_nc(dbg=_dbg)

    in_maps = []
    for c in range(NCORES):
        isl = slice(c * S, (c + 1) * S)
        m = {
            "wp0": _pack_tiles(wc[0][:, isl]),
            "wp1": _pack_tiles(wc[1][:, isl]),
            "xT": xT,
            "xisl": np.ascontiguousarray(xT[:, isl]),
            "wiT": wiT, "whT": whT, "WmT": WmT,
            "WaTnb": WaTnb, "WaTcur": WaTcur, "ba_col": ba_col,
            "biasr": biasr, "bhhr": bhhr, "hmask": hmask,
            "lnG": lnG, "lnB": lnB,
            "ones": np.ones((1, 128), f16),
            "ones2k": np.ones((1, N), f16),
            "onesf": np.ones((1, 64), np.float32),
            "ident": np.eye(128, dtype=f16),
            "onecol": np.ones((128, 128), f16),
            "zcol": np.zeros((1, 65), f16),
        }
        in_maps.append(m)

    import os
    trace = bool(int(os.environ.get("KERNEL_TRACE", "0")))
    if trace:
        try:
            import axon_ntff_shim  # noqa: F401  (registers the NTFF hook)
        except ImportError:
            trace = False
    res = run_bass_kernel_spmd(nc, in_maps, core_ids=list(range(NCORES)),
                               trace=trace)
    LAST_EXEC_NS = res.exec_time_ns
    out = np.concatenate([r["out"] for r in res.results], axis=0)
    if _dbg:
        global LAST_DBG
        LAST_DBG = res.results
    return out
